# revision 11
# baseline (speedup 1.0000x reference)
"""BERT-base 12-layer encoder forward on 8 trn2 NeuronCores.

Strategy: pure data parallelism — batch B=8, one sequence per core, full
weights replicated (bf16 in HBM, halving weight DMA), zero collectives.
All matmul operands (weights AND activations) are bf16 — the PE runs
bf16 at the same 1 column/cycle as fp32r, so this costs nothing on the
tensor engine while halving DMA and SBUF traffic; accumulation, LN
statistics, and softmax denominators stay fp32 in PSUM. Activations are
feature-major ([hidden, seq], hidden on SBUF partitions) so every
projection is a PE matmul with the weight stationary.

v2 changes vs baseline:
- QKV projections for head-pair p+1 are interleaved with the
  scores/exp/PV of pair p, so the Act engine's softmax-exp stream
  (~27us/layer, the attention bottleneck) hides behind the PE's
  projection matmuls instead of serializing after them.
- Softmax 1/denom uses reciprocal_approx_fast (1 DVE pass, ~18-bit)
  instead of the iterative-divide reciprocal (8 cyc/elem on a single
  partition lane).
- LayerNorm rstd = exp(-0.5*ln(var+eps)) on the Act engine: ln and exp
  live in the same activation-table set as the softmax exp
  (natural_log_exp_and_others), so the per-layer sqrt-set switches
  (~2.7us each, 2/layer) and the slow DVE reciprocal disappear.
"""
import sys

sys.path.insert(0, "/opt/trn_rl_repo")

import numpy as np
import concourse.bass as bass
import concourse.mybir as mybir
import concourse.tile as tile
from concourse import bacc
from concourse.bass_utils import run_bass_kernel_spmd

F32 = mybir.dt.float32
F32R = mybir.dt.float32r
BF16 = mybir.dt.bfloat16
AF = mybir.ActivationFunctionType
ALU = mybir.AluOpType

L, H, NH, I = 12, 768, 12, 3072
DH = 64
B, S = 8, 512
KT = H // 128          # 6 k-tiles over hidden
MT = H // 128          # 6 m-tiles over hidden
IT = I // 128          # 24 tiles over intermediate
ST = S // 128          # 4 tiles over sequence
NP = NH // 2           # 6 head pairs
EPS = 1e-12
SCALE = 0.125          # 1/sqrt(64)
RSQH = 1.0 / float(np.sqrt(H))
WDT = BF16             # weight dtype in HBM
ADT = BF16             # activation dtype (matmul operands)


def build_program(repeat=1, n_layers=L):
    nc = bacc.Bacc("TRN2", target_bir_lowering=False)

    XT = nc.dram_tensor("XT", [H, S], ADT, kind="ExternalInput")
    EXTM = nc.dram_tensor("EXTM", [ST, 128], F32, kind="ExternalInput")
    WQ = nc.dram_tensor("WQ", [L, H, H], WDT, kind="ExternalInput")
    WK = nc.dram_tensor("WK", [L, H, H], WDT, kind="ExternalInput")
    WV = nc.dram_tensor("WV", [L, H, H], WDT, kind="ExternalInput")
    WO = nc.dram_tensor("WO", [L, H, H], WDT, kind="ExternalInput")
    WI = nc.dram_tensor("WI", [L, IT, 128, KT, 128], WDT, kind="ExternalInput")
    WF = nc.dram_tensor("WF", [L, I, H], WDT, kind="ExternalInput")
    BVB = nc.dram_tensor("BVB", [L, H], ADT, kind="ExternalInput")
    BQ = nc.dram_tensor("BQ", [L, H], F32, kind="ExternalInput")
    BK = nc.dram_tensor("BK", [L, H], F32, kind="ExternalInput")
    BO = nc.dram_tensor("BO", [L, H], F32, kind="ExternalInput")
    BI = nc.dram_tensor("BI", [L, I], F32, kind="ExternalInput")
    BF = nc.dram_tensor("BF", [L, H], F32, kind="ExternalInput")
    G1 = nc.dram_tensor("G1", [L, H], F32, kind="ExternalInput")
    B1 = nc.dram_tensor("B1", [L, H], F32, kind="ExternalInput")
    G2 = nc.dram_tensor("G2", [L, H], F32, kind="ExternalInput")
    B2 = nc.dram_tensor("B2", [L, H], F32, kind="ExternalInput")
    OUT = nc.dram_tensor("OUT", [H, S], F32, kind="ExternalOutput")

    with tile.TileContext(nc) as tc:
        with (
            nc.allow_low_precision(reason="bf16 matmul pipeline"),
            tc.tile_pool(name="pers", bufs=1) as pers,
            tc.tile_pool(name="w768", bufs=26) as wpool,
            tc.tile_pool(name="wff1", bufs=4) as wf1pool,
            tc.tile_pool(name="sb", bufs=2) as sb,
        ):
            # ---- persistent activations ----
            xT = pers.tile([128, KT, S], ADT, tag="xT")
            nc.sync.dma_start(out=xT[:], in_=XT.ap().rearrange(
                "(k p) s -> p k s", p=128))
            qT = pers.tile([128, KT, S], ADT, tag="qT")    # reused as ctxT
            kTt = pers.tile([128, KT, S], ADT, tag="kTt")  # reused as LN input y
            attnT = pers.tile([128, KT, S], ADT, tag="attnT")
            v_aug = pers.tile([128, ST, NH, DH + 1], ADT, tag="vaug")
            nc.vector.memset(v_aug[:, :, :, DH], 1.0)

            ext = pers.tile([128, ST], F32, tag="ext")
            nc.sync.dma_start(out=ext[:], in_=EXTM.ap().rearrange("k p -> p k"))

            # ---- constants ----
            ones128c = pers.tile([1, 128], F32, tag="ones128c")
            nc.vector.memset(ones128c[:], 1.0)
            invh128c = pers.tile([1, 128], F32, tag="invh128c")
            nc.vector.memset(invh128c[:], 1.0 / H)
            ones128p = pers.tile([128, 1], ADT, tag="ones128p")
            nc.vector.memset(ones128p[:], 1.0)
            ones128pc = pers.tile([1, 128], ADT, tag="ones128pc")
            nc.vector.memset(ones128pc[:], 1.0)
            ones64 = pers.tile([1, DH], F32, tag="ones64")
            nc.vector.memset(ones64[:], 1.0)
            eps_t = pers.tile([1, 1], F32, tag="eps")
            nc.vector.memset(eps_t[:], EPS)

            # ---- per-layer params, loaded once (feature-major [128, L, KT]) ----
            def ppar(name, dram, kt):
                t = pers.tile([128, L, kt], F32, tag=name, name=name)
                nc.sync.dma_start(out=t[:], in_=dram.ap().rearrange(
                    "l (k p) -> p l k", p=128))
                return t
            bq_t = ppar("bq", BQ, KT); bk_t = ppar("bk", BK, KT)
            bo_t = ppar("bo", BO, KT); bf_t = ppar("bf", BF, KT)
            g1_t = ppar("g1", G1, KT); b1_t = ppar("b1", B1, KT)
            g2_t = ppar("g2", G2, KT); b2_t = ppar("b2", B2, KT)
            bi_t = ppar("bi", BI, IT)

            def ln_sums(ps, y, k, first, last, st_ps):
                """Accumulate sum / sumsq of y k-tile into st_ps rows.
                Squares ride the idle GPSIMD engine so the Act engine's
                queue stays clear for the stats chain that follows."""
                nc.tensor.matmul(st_ps[:, 0, :], ones128p[:], y[:, k, :],
                                 start=first, stop=last)
                sq = sb.tile([128, S], ADT, tag="sq", name="sq", bufs=4)
                nc.gpsimd.tensor_mul(sq[:], y[:, k, :], y[:, k, :])
                nc.tensor.matmul(st_ps[:, 1, :], ones128p[:], sq[:],
                                 start=first, stop=last)

            def ln_sums_split(ps, y, st_ps):
                """Sums in separate pipelined loops (for a phase tail where
                the PE has no other work to hide per-tile latencies)."""
                for k in range(KT):
                    nc.tensor.matmul(st_ps[:, 0, :], ones128p[:], y[:, k, :],
                                     start=(k == 0), stop=(k == KT - 1))
                for k in range(KT):
                    sq = sb.tile([128, S], ADT, tag="sq", name="sq", bufs=4)
                    nc.gpsimd.tensor_mul(sq[:], y[:, k, :], y[:, k, :])
                    nc.tensor.matmul(st_ps[:, 1, :], ones128p[:], sq[:],
                                     start=(k == 0), stop=(k == KT - 1))

            def ln_finalize(ps, y, gam, bet, l, out, st_ps, warm=0):
                """LN stats + normalize.

                rstd = exp(-0.5*ln(varh/H + eps)) keeps the whole chain in
                the natural_log_exp activation-table set (no sqrt-set
                switch, no slow DVE reciprocal).

                `warm` > 0 issues dummy ones-matmuls into a scratch PSUM
                bank while the stats chain runs — the PE p-state drops to
                1.2GHz after a ~3.4us idle gap, so keeping it streaming
                through this serial chain makes the next phase's matmuls
                start at full clock."""
                if warm:
                    w_ps = ps.tile([1, S], F32, tag="warm", name="warm")
                    for w in range(warm):
                        nc.tensor.matmul(w_ps[:], ones128p[:], y[:, w % KT, :],
                                         start=True, stop=True)
                sums = sb.tile([1, S], F32, tag="sums", name="sums", bufs=2)
                nc.vector.tensor_copy(sums[:], st_ps[:, 0, :])
                # s2h = (sum/sqrt(H))^2 straight from PSUM on Act
                s2h = sb.tile([1, S], F32, tag="s2h", name="s2h", bufs=2)
                nc.scalar.activation(s2h[:], st_ps[:, 0, :], AF.Square,
                                     scale=RSQH)
                varh = sb.tile([1, S], F32, tag="varh", name="varh", bufs=2)
                nc.vector.tensor_sub(varh[:], st_ps[:, 1, :], s2h[:])
                lnv = sb.tile([1, S], F32, tag="lnv", name="lnv", bufs=2)
                nc.scalar.activation(lnv[:], varh[:], AF.Ln,
                                     bias=eps_t[:], scale=1.0 / H)
                rstd = sb.tile([1, S], F32, tag="rstd", name="rstd", bufs=2)
                nc.scalar.activation(rstd[:], lnv[:], AF.Exp, scale=-0.5)
                mub_ps = ps.tile([128, S], F32, tag="mub", name="mub", bufs=1)
                nc.tensor.matmul(mub_ps[:], invh128c[:].bitcast(F32R),
                                 sums[:].bitcast(F32R),
                                 start=True, stop=True)
                rsb_ps = ps.tile([128, S], F32, tag="rsb", name="rsb", bufs=1)
                nc.tensor.matmul(rsb_ps[:], ones128c[:].bitcast(F32R),
                                 rstd[:].bitcast(F32R),
                                 start=True, stop=True)
                if warm:
                    for w in range(warm):
                        nc.tensor.matmul(w_ps[:], ones128p[:], y[:, w % KT, :],
                                         start=True, stop=True)
                # SBUF copies of the broadcasts so the (otherwise idle)
                # GPSIMD engine can handle half the normalize tiles — it has
                # no PSUM port. DVE tiles keep reading PSUM directly so the
                # first output tile doesn't wait for the copies.
                mub_sb = sb.tile([128, S], F32, tag="mubsb", name="mubsb", bufs=2)
                nc.scalar.copy(mub_sb[:], mub_ps[:])
                rsb_sb = sb.tile([128, S], F32, tag="rsbsb", name="rsbsb", bufs=2)
                nc.scalar.copy(rsb_sb[:], rsb_ps[:])
                for k in range(KT):
                    dve = k % 2 == 0
                    eng = nc.vector if dve else nc.gpsimd
                    mu_in = mub_ps[:] if dve else mub_sb[:]
                    rs_in = rsb_ps[:] if dve else rsb_sb[:]
                    t1 = sb.tile([128, S], F32, tag="lnt1", name="lnt1", bufs=4)
                    eng.tensor_sub(t1[:], y[:, k, :], mu_in)
                    t2 = sb.tile([128, S], F32, tag="lnt2", name="lnt2", bufs=4)
                    eng.tensor_mul(t2[:], t1[:], rs_in)
                    nc.scalar.activation(out[:, k, :], t2[:], AF.Identity,
                                         bias=bet[:, l, k:k + 1],
                                         scale=gam[:, l, k:k + 1])

            def load_w768(dram, l, k, name):
                w = wpool.tile([128, H], WDT, tag="w768", name=name)
                nc.sync.dma_start(out=w[:], in_=dram.ap()[l, bass.ts(k, 128), :])
                return w

            def layer_body(l):
                # ============ QKV + attention, pair-pipelined ============
                # Head-pair m needs exactly the m-th 128-wide output tile of
                # the Q and K projections, so those are computed per-pair and
                # interleaved with the previous pair's softmax: while the Act
                # engine streams exp tiles for pair p, the PE runs pair
                # p+1's Q/K projections (and V / PV work). The PE never
                # waits on the Act exp stream and vice versa.
                wq = [load_w768(WQ, l, k, f"wq{k}") for k in range(KT)]
                wk = [load_w768(WK, l, k, f"wk{k}") for k in range(KT)]
                wv = [load_w768(WV, l, k, f"wv{k}") for k in range(KT)]

                def proj_qk(ps, m):
                    p_q = ps.tile([128, S], F32, tag="qk", name="pq", bufs=2)
                    for k in range(KT):
                        nc.tensor.matmul(p_q[:], wq[k][:, bass.ts(m, 128)],
                                         xT[:, k, :], start=(k == 0),
                                         stop=(k == KT - 1))
                    nc.scalar.activation(qT[:, m, :], p_q[:], AF.Identity,
                                         bias=bq_t[:, l, m:m + 1])
                    p_k = ps.tile([128, S], F32, tag="qk", name="pk", bufs=2)
                    for k in range(KT):
                        nc.tensor.matmul(p_k[:], wk[k][:, bass.ts(m, 128)],
                                         xT[:, k, :], start=(k == 0),
                                         stop=(k == KT - 1))
                    nc.scalar.activation(kTt[:, m, :], p_k[:], AF.Identity,
                                         bias=bk_t[:, l, m:m + 1])

                def scores_exp(ps, pr):
                    exps = {}
                    for hh in range(2):
                        p0 = hh * DH
                        tp = None if hh == 0 else (64, 0)
                        for m in range(ST):
                            s_ps = ps.tile([128, S], F32, tag="scores",
                                           name="sps", bufs=2)
                            nc.tensor.matmul(
                                s_ps[:],
                                kTt[p0:p0 + DH, pr, bass.ts(m, 128)],
                                qT[p0:p0 + DH, pr, :],
                                start=True, stop=True, tile_position=tp)
                            e_t = sb.tile([128, S], ADT, tag="exp",
                                          name="expt", bufs=16)
                            nc.scalar.activation(e_t[:], s_ps[:], AF.Exp,
                                                 bias=ext[:, m:m + 1],
                                                 scale=SCALE)
                            exps[(hh, m)] = e_t
                    return exps

                def pv(ps, pr, exps):
                    rcp2 = sb.tile([1, 2, S], F32, tag="rcp2", name="rcp2",
                                   bufs=3)
                    c_pss = []
                    for hh in range(2):
                        h = 2 * pr + hh
                        c_ps = ps.tile([128, S], F32, tag="ctx", name="cps",
                                       bufs=4)
                        for m in range(ST):
                            nc.tensor.matmul(c_ps[0:DH + 1, :],
                                             v_aug[:, m, h, :],
                                             exps[(hh, m)][:],
                                             start=(m == 0), stop=(m == ST - 1))
                        nc.vector.reciprocal_approx_fast(
                            out=rcp2[:, hh, :], in_=c_ps[DH:DH + 1, :])
                        c_pss.append(c_ps)
                    return (pr, c_pss, rcp2)

                def att_finalize(ps, pend):
                    # bc_ps shares the scores pool rotation (no extra bank)
                    pr_, c_pss_, rcp2_ = pend
                    bc_ps = ps.tile([128, S], F32, tag="scores", name="bcps",
                                    bufs=2)
                    for hh in range(2):
                        nc.tensor.matmul(bc_ps[hh * DH:(hh + 1) * DH, :],
                                         ones64[:].bitcast(F32R),
                                         rcp2_[:, hh, :].bitcast(F32R),
                                         start=True, stop=True)
                    bc_sb = sb.tile([128, S], ADT, tag="bcsb", name="bcsb",
                                    bufs=2)
                    nc.vector.tensor_copy(bc_sb[:], bc_ps[:])
                    for hh in range(2):
                        p0 = hh * DH
                        nc.vector.tensor_mul(
                            qT[p0:p0 + DH, pr_, :],
                            c_pss_[hh][0:DH, :],
                            bc_sb[p0:p0 + DH, :])

                with tc.tile_pool(name="ps_qk", bufs=1, space="PSUM") as psqk:
                    # pair 0 projections + scores, then V while its exps run
                    proj_qk(psqk, 0)
                    with tc.tile_pool(name="ps_sc", bufs=1,
                                      space="PSUM") as pssc:
                        exps0 = scores_exp(pssc, 0)
                        # V projection (seq-major, bias via K=1 ones matmul)
                        bv_row = sb.tile([1, H], ADT, tag="bvrow",
                                         name="bvrow", bufs=2)
                        nc.sync.dma_start(out=bv_row[:], in_=BVB.ap()[l:l + 1, :])
                        with tc.tile_pool(name="ps_v", bufs=1,
                                          space="PSUM") as psv:
                            for s in range(ST):
                                p_a = psv.tile([128, S], F32, tag="va",
                                               name="pva", bufs=2)
                                p_b = psv.tile([128, 256], F32, tag="vb",
                                               name="pvb", bufs=2)
                                for k in range(KT):
                                    nc.tensor.matmul(
                                        p_a[:], xT[:, k, bass.ts(s, 128)],
                                        wv[k][:, 0:512],
                                        start=(k == 0), stop=False)
                                    nc.tensor.matmul(
                                        p_b[:], xT[:, k, bass.ts(s, 128)],
                                        wv[k][:, 512:768],
                                        start=(k == 0), stop=False)
                                nc.tensor.matmul(p_a[:], ones128pc[:],
                                                 bv_row[:, 0:512],
                                                 start=False, stop=True)
                                nc.tensor.matmul(p_b[:], ones128pc[:],
                                                 bv_row[:, 512:768],
                                                 start=False, stop=True)
                                nc.vector.tensor_copy(
                                    v_aug[:, s, 0:8, 0:DH],
                                    p_a[:].rearrange("p (h c) -> p h c", c=DH))
                                nc.vector.tensor_copy(
                                    v_aug[:, s, 8:12, 0:DH],
                                    p_b[:].rearrange("p (h c) -> p h c", c=DH))
                        # steady pair pipeline
                        with tc.tile_pool(name="ps_ctx", bufs=1,
                                          space="PSUM") as psctx:
                            pending = None
                            exps = exps0
                            for pr in range(NP):
                                if pr + 1 < NP:
                                    proj_qk(psqk, pr + 1)
                                # finalize pair pr-1 now: its reciprocal was
                                # issued a full pair-iteration ago, so the
                                # broadcast matmuls at the PE queue head
                                # never wait on the DVE chain — and its
                                # scores-pool slot was drained by exps(pr)
                                # long ago.
                                if pending is not None:
                                    att_finalize(pssc, pending)
                                nxt = (scores_exp(pssc, pr + 1)
                                       if pr + 1 < NP else None)
                                pending = pv(psctx, pr, exps)
                                exps = nxt
                            att_finalize(pssc, pending)

                # ================= Wo + residual + LN1 =================
                with tc.tile_pool(name="ps_wo", bufs=1, space="PSUM") as ps:
                    wo = [load_w768(WO, l, k, f"wo{k}") for k in range(KT)]
                    st_ps = ps.tile([1, 2, S], F32, tag="sum", name="sum")
                    for m in range(MT):
                        p_o = ps.tile([128, S], F32, tag="proj", name="po",
                                      bufs=3)
                        for k in range(KT):
                            nc.tensor.matmul(p_o[:], wo[k][:, bass.ts(m, 128)],
                                             qT[:, k, :], start=(k == 0),
                                             stop=(k == KT - 1))
                        # y = (psum + bo) + x   (into kTt, reused as y)
                        nc.vector.scalar_tensor_tensor(
                            kTt[:, m, :], p_o[:], bo_t[:, l, m:m + 1],
                            xT[:, m, :], op0=ALU.add, op1=ALU.add)
                        ln_sums(ps, kTt, m, m == 0, m == MT - 1, st_ps)
                    ln_finalize(ps, kTt, g1_t, b1_t, l, attnT, st_ps, warm=6)

                # ================= FFN =================
                with tc.tile_pool(name="ps_ffn", bufs=1, space="PSUM") as ps:
                    ffo = [ps.tile([128, S], F32, tag="ffo", name=f"ffo{m}",
                                   bufs=6)
                           for m in range(MT)]
                    # software-pipelined: p_f chain of ko+1 issues before the
                    # ffo accumulation of ko so the PE isn't starved while the
                    # gelu of ko drains.
                    ffts = {}
                    for ko in range(IT + 1):
                        if ko < IT:
                            wi_t = wf1pool.tile([128, KT, 128], WDT, tag="wff1",
                                                name=f"wi{ko}")
                            nc.sync.dma_start(out=wi_t[:], in_=WI.ap()[l, ko])
                            wf_t = wpool.tile([128, H], WDT, tag="w768",
                                              name=f"wf{ko}")
                            nc.sync.dma_start(out=wf_t[:],
                                              in_=WF.ap()[l, bass.ts(ko, 128), :])
                            p_f = ps.tile([128, S], F32, tag="ff1", name="pf",
                                          bufs=2)
                            for k in range(KT):
                                nc.tensor.matmul(p_f[:], wi_t[:, k, :],
                                                 attnT[:, k, :],
                                                 start=(k == 0),
                                                 stop=(k == KT - 1))
                            ff_t = sb.tile([128, S], ADT, tag="fft", name="fft",
                                           bufs=4)
                            nc.scalar.activation(ff_t[:], p_f[:], AF.Gelu,
                                                 bias=bi_t[:, l, ko:ko + 1])
                            ffts[ko] = (ff_t, wf_t)
                        if ko >= 1:
                            ff_p, wf_p = ffts.pop(ko - 1)
                            for m in range(MT):
                                nc.tensor.matmul(ffo[m][:],
                                                 wf_p[:, bass.ts(m, 128)],
                                                 ff_p[:], start=(ko - 1 == 0),
                                                 stop=(ko - 1 == IT - 1))
                    for m in range(MT):
                        # y2 = (ffo + bf) + attnT   (into kTt)
                        nc.vector.scalar_tensor_tensor(
                            kTt[:, m, :], ffo[m][:], bf_t[:, l, m:m + 1],
                            attnT[:, m, :], op0=ALU.add, op1=ALU.add)
                with tc.tile_pool(name="ps_ln2", bufs=1, space="PSUM") as ps:
                    st_ps = ps.tile([1, 2, S], F32, tag="sum", name="sum")
                    ln_sums_split(ps, kTt, st_ps)
                    ln_finalize(ps, kTt, g2_t, b2_t, l, xT, st_ps, warm=8)

            for _ in range(repeat):
                for l in range(n_layers):
                    layer_body(l)

            xout = pers.tile([128, KT, S], F32, tag="xout")
            nc.vector.tensor_copy(xout[:], xT[:])
            nc.sync.dma_start(
                out=OUT.ap().rearrange("(k p) s -> p k s", p=128),
                in_=xout[:])

    nc.compile()
    return nc


_CACHE = {}


def get_program(repeat=1, n_layers=L):
    key = (repeat, n_layers)
    if key not in _CACHE:
        _CACHE[key] = build_program(repeat, n_layers)
    return _CACHE[key]


def make_input_maps(inputs):
    """Per-core input maps from the full-batch input dict."""
    import ml_dtypes
    wnp = ml_dtypes.bfloat16 if WDT == BF16 else np.float32
    anp = ml_dtypes.bfloat16 if ADT == BF16 else np.float32
    hs = np.ascontiguousarray(np.asarray(inputs["hidden_states"], np.float32))
    mask = np.asarray(inputs["attention_mask"], np.float32)
    wi = np.ascontiguousarray(
        np.asarray(inputs["Wi"], np.float32).reshape(L, KT, 128, IT, 128)
        .transpose(0, 3, 2, 1, 4)).astype(wnp)
    shared = {
        "WQ": np.ascontiguousarray(np.asarray(inputs["Wq"], np.float32)).astype(wnp),
        "WK": np.ascontiguousarray(np.asarray(inputs["Wk"], np.float32)).astype(wnp),
        "WV": np.ascontiguousarray(np.asarray(inputs["Wv"], np.float32)).astype(wnp),
        "WO": np.ascontiguousarray(np.asarray(inputs["Wo"], np.float32)).astype(wnp),
        "WI": wi,
        "WF": np.ascontiguousarray(np.asarray(inputs["Wf"], np.float32)).astype(wnp),
        "BVB": np.asarray(inputs["bv"], np.float32).astype(anp),
        "BQ": np.asarray(inputs["bq"], np.float32),
        "BK": np.asarray(inputs["bk"], np.float32),
        "BO": np.asarray(inputs["bo"], np.float32),
        "BI": np.asarray(inputs["bi"], np.float32),
        "BF": np.asarray(inputs["bf"], np.float32),
        "G1": np.asarray(inputs["ln1_g"], np.float32),
        "B1": np.asarray(inputs["ln1_b"], np.float32),
        "G2": np.asarray(inputs["ln2_g"], np.float32),
        "B2": np.asarray(inputs["ln2_b"], np.float32),
    }
    in_maps = []
    for c in range(B):
        ext = ((1.0 - mask[c]) * -10000.0).astype(np.float32).reshape(ST, 128)
        in_maps.append({
            "XT": np.ascontiguousarray(hs[c].T).astype(anp),
            "EXTM": ext,
            **shared,
        })
    return in_maps


def kernel(**inputs):
    nc = get_program(repeat=1)
    in_maps = make_input_maps(inputs)
    res = run_bass_kernel_spmd(nc, in_maps, list(range(B)))
    out = np.stack([res.results[c]["OUT"].T for c in range(B)], axis=0)
    return out.astype(np.float32)


# revision 60
# speedup vs baseline: 1485.7135x; 1485.7135x over previous
"""BERT-base 12-layer encoder forward on 8 trn2 NeuronCores.

Strategy: pure data parallelism — batch B=8, one sequence per core, full
weights replicated (bf16 in HBM, halving weight DMA), zero collectives.
All matmul operands (weights AND activations) are bf16 — the PE runs
bf16 at the same 1 column/cycle as fp32r, so this costs nothing on the
tensor engine while halving DMA and SBUF traffic; accumulation, LN
statistics, and softmax denominators stay fp32 in PSUM. Activations are
feature-major ([hidden, seq], hidden on SBUF partitions) so every
projection is a PE matmul with the weight stationary.

v2 changes vs baseline:
- QKV projections for head-pair p+1 are interleaved with the
  scores/exp/PV of pair p, so the Act engine's softmax-exp stream
  (~27us/layer, the attention bottleneck) hides behind the PE's
  projection matmuls instead of serializing after them.
- Softmax 1/denom uses reciprocal_approx_fast (1 DVE pass, ~18-bit)
  instead of the iterative-divide reciprocal (8 cyc/elem on a single
  partition lane).
- LayerNorm rstd = exp(-0.5*ln(var+eps)) on the Act engine: ln and exp
  live in the same activation-table set as the softmax exp
  (natural_log_exp_and_others), so the per-layer sqrt-set switches
  (~2.7us each, 2/layer) and the slow DVE reciprocal disappear.
"""
import sys

sys.path.insert(0, "/opt/trn_rl_repo")

import numpy as np
import concourse.bass as bass
import concourse.mybir as mybir
import concourse.tile as tile
from concourse import bacc
from concourse.bass_utils import run_bass_kernel_spmd
from concourse.dve_ops import RECIPROCAL_APPROX_FAST, RECIP_APPROX_FAST_CONSTS

F32 = mybir.dt.float32
F32R = mybir.dt.float32r
BF16 = mybir.dt.bfloat16
AF = mybir.ActivationFunctionType
ALU = mybir.AluOpType

L, H, NH, I = 12, 768, 12, 3072
DH = 64
B, S = 8, 512
KT = H // 128          # 6 k-tiles over hidden
MT = H // 128          # 6 m-tiles over hidden
IT = I // 128          # 24 tiles over intermediate
ST = S // 128          # 4 tiles over sequence
NP = NH // 2           # 6 head pairs
EPS = 1e-12
SCALE = 0.125          # 1/sqrt(64)
RSQH = 1.0 / float(np.sqrt(H))
WDT = BF16             # weight dtype in HBM
ADT = BF16             # activation dtype (matmul operands)


def build_program(repeat=1, n_layers=L):
    # Build-time activation-table steering: the default greedy set picker
    # chooses exp_and_others for Exp and natural_log for Ln, costing 6
    # ACT_TABLE_LOADs (~2.7us each on HW) per layer. Restricting the
    # visible sets to natural_log_exp_and_others (exp+ln+square+identity)
    # and gelu_and_others makes every layer need exactly 2 loads
    # (exp-set <-> gelu-set around the FFN). Indices are preserved, so the
    # emitted act_func_set_id still refers to the real act_info.json
    # entries. Restored right after the build.
    import concourse.hw_specs as hw_specs
    _orig_gat = hw_specs.get_activation_tables
    _keep = {"natural_log_exp_and_others", "gelu_and_others"}

    def _patched_gat(module_arch):
        tabs = _orig_gat(module_arch)
        return {name: (fns if name in _keep else set())
                for name, fns in tabs.items()}

    hw_specs.get_activation_tables = _patched_gat
    import concourse.bacc as bacc_mod
    _bacc_had = getattr(bacc_mod, "get_activation_tables", None)
    if _bacc_had is not None:
        bacc_mod.get_activation_tables = _patched_gat
    try:
        return _build_program_inner(repeat, n_layers)
    finally:
        hw_specs.get_activation_tables = _orig_gat
        if _bacc_had is not None:
            bacc_mod.get_activation_tables = _bacc_had


def _build_program_inner(repeat=1, n_layers=L):
    nc = bacc.Bacc("TRN2", target_bir_lowering=False)

    XT = nc.dram_tensor("XT", [H, S], ADT, kind="ExternalInput")
    EXTM = nc.dram_tensor("EXTM", [ST, 128], F32, kind="ExternalInput")
    WQ = nc.dram_tensor("WQ", [L, H, H], WDT, kind="ExternalInput")
    WK = nc.dram_tensor("WK", [L, H, H], WDT, kind="ExternalInput")
    WV = nc.dram_tensor("WV", [L, H, H], WDT, kind="ExternalInput")
    WO = nc.dram_tensor("WO", [L, H, H], WDT, kind="ExternalInput")
    WI = nc.dram_tensor("WI", [L, IT, 128, KT, 128], WDT, kind="ExternalInput")
    WF = nc.dram_tensor("WF", [L, I, H], WDT, kind="ExternalInput")
    BVB = nc.dram_tensor("BVB", [L, H], ADT, kind="ExternalInput")
    # 8 per-layer param vectors pre-transposed host-side to the on-chip
    # feature-major layout [128, L, KT] and packed into one contiguous
    # tensor (one big DMA instead of 9 scatter-pattern DMAs with ~72
    # descriptors per partition each).
    PP8 = nc.dram_tensor("PP8", [128, 8, L, KT], F32, kind="ExternalInput")
    PBI = nc.dram_tensor("PBI", [128, L, IT], F32, kind="ExternalInput")
    OUT = nc.dram_tensor("OUT", [H, S], F32, kind="ExternalOutput")
    import os
    KDBG = os.environ.get("KDBG") == "1"
    if KDBG:
        DQ = nc.dram_tensor("DQ", [H, S], F32, kind="ExternalOutput")
        DK = nc.dram_tensor("DK", [H, S], F32, kind="ExternalOutput")

    with tile.TileContext(nc) as tc:
        with (
            nc.allow_low_precision(reason="bf16 matmul pipeline"),
            tc.tile_pool(name="pers", bufs=1) as pers,
            tc.tile_pool(name="w768", bufs=26) as wpool,
            tc.tile_pool(name="wff1", bufs=4) as wf1pool,
            tc.tile_pool(name="sb", bufs=2) as sb,
        ):
            # ---- persistent activations ----
            xT = pers.tile([128, KT, S], ADT, tag="xT")
            nc.sync.dma_start(out=xT[:], in_=XT.ap().rearrange(
                "(k p) s -> p k s", p=128))
            qT = pers.tile([128, KT, S], ADT, tag="qT")    # reused as ctxT
            kTt = pers.tile([128, KT, S], ADT, tag="kTt")  # reused as LN input y
            attnT = pers.tile([128, KT, S], ADT, tag="attnT")
            v_aug = pers.tile([128, ST, NH, DH + 1], ADT, tag="vaug")
            nc.vector.memset(v_aug[:, :, :, DH], 1.0)

            ext = pers.tile([128, ST], F32, tag="ext")
            nc.sync.dma_start(out=ext[:], in_=EXTM.ap().rearrange("k p -> p k"))

            # ---- constants ----
            ones128c = pers.tile([1, 128], ADT, tag="ones128c")
            nc.vector.memset(ones128c[:], 1.0)
            invh128c = pers.tile([1, 128], ADT, tag="invh128c")
            nc.vector.memset(invh128c[:], 1.0 / H)
            ones128p = pers.tile([128, 1], ADT, tag="ones128p")
            nc.vector.memset(ones128p[:], 1.0)
            ones128pc = pers.tile([1, 128], ADT, tag="ones128pc")
            nc.vector.memset(ones128pc[:], 1.0)
            ones64 = pers.tile([1, DH], ADT, tag="ones64")
            nc.vector.memset(ones64[:], 1.0)
            eps_t = pers.tile([1, 1], F32, tag="eps")
            nc.vector.memset(eps_t[:], EPS)

            # ---- per-layer params, loaded once (feature-major [128, L, KT],
            # pre-transposed host-side; single contiguous DMA) ----
            pp8 = pers.tile([128, 8, L, KT], F32, tag="pp8")
            nc.sync.dma_start(out=pp8[:], in_=PP8.ap())
            bi_t = pers.tile([128, L, IT], F32, tag="pbi")
            nc.sync.dma_start(out=bi_t[:], in_=PBI.ap())
            bq_t = pp8[:, 0]; bk_t = pp8[:, 1]
            bo_t = pp8[:, 2]; bf_t = pp8[:, 3]
            g1_t = pp8[:, 4]; b1_t = pp8[:, 5]
            g2_t = pp8[:, 6]; b2_t = pp8[:, 7]

            def ln_sums(ps, y, k, first, last, st_ps):
                """Accumulate sum / sumsq of y k-tile into st_ps rows.
                Squares ride the idle GPSIMD engine so the Act engine's
                queue stays clear for the stats chain that follows."""
                nc.tensor.matmul(st_ps[:, 0, :], ones128p[:], y[:, k, :],
                                 start=first, stop=last)
                sq = sb.tile([128, S], ADT, tag="sq", name="sq", bufs=4)
                nc.gpsimd.tensor_mul(sq[:], y[:, k, :], y[:, k, :])
                nc.tensor.matmul(st_ps[:, 1, :], ones128p[:], sq[:],
                                 start=first, stop=last)

            def ln_sums_split(ps, y, st_ps):
                """Sums in separate pipelined loops (for a phase tail where
                the PE has no other work to hide per-tile latencies).
                Squares alternate gpsimd/Act: the Act engine is idle after
                the last gelu, and gpsimd alone was the ~6.4us long pole."""
                for k in range(KT):
                    nc.tensor.matmul(st_ps[:, 0, :], ones128p[:], y[:, k, :],
                                     start=(k == 0), stop=(k == KT - 1))
                for k in range(KT):
                    sq = sb.tile([128, S], ADT, tag="sq", name="sq", bufs=4)
                    if k % 2 == 0:
                        nc.gpsimd.tensor_mul(sq[:], y[:, k, :], y[:, k, :])
                    else:
                        nc.scalar.activation(sq[:], y[:, k, :], AF.Square)
                    nc.tensor.matmul(st_ps[:, 1, :], ones128p[:], sq[:],
                                     start=(k == 0), stop=(k == KT - 1))

            def ln_finalize(ps, y, gam, bet, l, out, st_ps, warm=0,
                            preload_gelu=False):
                """LN stats + normalize.

                rstd = exp(-0.5*ln(varh/H + eps)) keeps the whole chain in
                the natural_log_exp activation-table set (no sqrt-set
                switch, no slow DVE reciprocal).

                `warm` > 0 issues dummy ones-matmuls into a scratch PSUM
                bank while the stats chain runs — the PE p-state drops to
                1.2GHz after a ~3.4us idle gap, so keeping it streaming
                through this serial chain makes the next phase's matmuls
                start at full clock."""
                if warm:
                    w_ps = ps.tile([1, S], F32, tag="warm", name="warm")
                    for w in range(warm):
                        nc.tensor.matmul(w_ps[:], ones128p[:], y[:, w % KT, :],
                                         start=True, stop=True)
                sums = sb.tile([1, S], ADT, tag="sums", name="sums", bufs=2)
                nc.vector.tensor_copy(sums[:], st_ps[:, 0, :])
                # s2h = (sum/sqrt(H))^2 straight from PSUM on Act
                s2h = sb.tile([1, S], F32, tag="s2h", name="s2h", bufs=2)
                nc.scalar.activation(s2h[:], st_ps[:, 0, :], AF.Square,
                                     scale=RSQH)
                varh = sb.tile([1, S], F32, tag="varh", name="varh", bufs=2)
                nc.vector.tensor_sub(varh[:], st_ps[:, 1, :], s2h[:])
                lnv = sb.tile([1, S], F32, tag="lnv", name="lnv", bufs=2)
                nc.scalar.activation(lnv[:], varh[:], AF.Ln,
                                     bias=eps_t[:], scale=1.0 / H)
                rstd = sb.tile([1, S], ADT, tag="rstd", name="rstd", bufs=2)
                nc.scalar.activation(rstd[:], lnv[:], AF.Exp, scale=-0.5)
                if preload_gelu:
                    # dummy 1-elem gelu: forces the gelu-set ACT_TABLE_LOAD
                    # here (Act has slack during the normalize wave) instead
                    # of on the FFN critical path. The normalize identities
                    # that follow exist in every set, so they need no
                    # reload.
                    scr = sb.tile([1, 1], F32, tag="scr", name="scr", bufs=2)
                    nc.scalar.activation(scr[:], eps_t[:], AF.Gelu)
                mub_ps = ps.tile([128, S], F32, tag="mub", name="mub", bufs=1)
                nc.tensor.matmul(mub_ps[:], invh128c[:], sums[:],
                                 start=True, stop=True)
                rsb_ps = ps.tile([128, S], F32, tag="rsb", name="rsb", bufs=1)
                nc.tensor.matmul(rsb_ps[:], ones128c[:], rstd[:],
                                 start=True, stop=True)
                if warm:
                    for w in range(warm):
                        nc.tensor.matmul(w_ps[:], ones128p[:], y[:, w % KT, :],
                                         start=True, stop=True)
                # bf16 SBUF copies of the broadcasts: the GPSIMD half needs
                # SBUF (no PSUM port), and bf16 operands put the DVE
                # tensor_tensor ops in 2x mode (424ns vs 690ns a tile).
                # Mean/rstd in bf16 cost ~1e-3 relative — well inside
                # tolerance. DVE tile 0 reads PSUM directly so the first
                # output tile doesn't wait for the copies.
                mub_sb = sb.tile([128, S], ADT, tag="mubsb", name="mubsb", bufs=2)
                nc.scalar.copy(mub_sb[:], mub_ps[:])
                rsb_sb = sb.tile([128, S], ADT, tag="rsbsb", name="rsbsb", bufs=2)
                nc.scalar.copy(rsb_sb[:], rsb_ps[:])
                for k in range(KT):
                    dve = k % 2 == 0
                    eng = nc.vector if dve else nc.gpsimd
                    first = dve and k == 0
                    mu_in = mub_ps[:] if first else mub_sb[:]
                    rs_in = rsb_ps[:] if first else rsb_sb[:]
                    t1 = sb.tile([128, S], F32 if first else ADT, tag="lnt1",
                                 name="lnt1", bufs=4)
                    eng.tensor_sub(t1[:], y[:, k, :], mu_in)
                    t2 = sb.tile([128, S], ADT, tag="lnt2", name="lnt2", bufs=4)
                    eng.tensor_mul(t2[:], t1[:], rs_in)
                    nc.scalar.activation(out[:, k, :], t2[:], AF.Identity,
                                         bias=bet[:, l, k:k + 1],
                                         scale=gam[:, l, k:k + 1])

            def load_w768(dram, l, k, name):
                w = wpool.tile([128, H], WDT, tag="w768", name=name)
                nc.sync.dma_start(out=w[:], in_=dram.ap()[l, bass.ts(k, 128), :])
                return w

            def layer_body(l, last=False):
                # ============ QKV + attention, pair-pipelined ============
                # Q is projected first in two k-outer groups of 3 m-tiles
                # (3 PSUM banks): the k-outer order lets each matmul start
                # as soon as the previous layer's LN2 normalize emits that
                # xT k-tile, filling the LN2 chain's PE-idle window.
                # Head-pair m then needs exactly the m-th K-projection
                # tile, so K is computed per-pair inside the softmax
                # pipeline: while the Act engine streams exp tiles for
                # pair p+1, the PE runs K_{p+1} and PV_p.
                wq = [load_w768(WQ, l, k, f"wq{k}") for k in range(KT)]
                wk = [load_w768(WK, l, k, f"wk{k}") for k in range(KT)]
                wv = [load_w768(WV, l, k, f"wv{k}") for k in range(KT)]

                def proj_qk(ps, m):
                    p_q = ps.tile([128, S], F32, tag="qk", name="pq", bufs=2)
                    for k in range(KT):
                        nc.tensor.matmul(p_q[:], wq[k][:, bass.ts(m, 128)],
                                         xT[:, k, :], start=(k == 0),
                                         stop=(k == KT - 1))
                    nc.scalar.activation(qT[:, m, :], p_q[:], AF.Identity,
                                         bias=bq_t[:, l, m:m + 1])
                    p_k = ps.tile([128, S], F32, tag="qk", name="pk", bufs=2)
                    for k in range(KT):
                        nc.tensor.matmul(p_k[:], wk[k][:, bass.ts(m, 128)],
                                         xT[:, k, :], start=(k == 0),
                                         stop=(k == KT - 1))
                    nc.scalar.activation(kTt[:, m, :], p_k[:], AF.Identity,
                                         bias=bk_t[:, l, m:m + 1])

                def scores_exp(ps, pr):
                    exps = {}
                    for hh in range(2):
                        p0 = hh * DH
                        tp = None if hh == 0 else (64, 0)
                        for m in range(ST):
                            s_ps = ps.tile([128, S], F32, tag="scores",
                                           name="sps", bufs=2)
                            nc.tensor.matmul(
                                s_ps[:],
                                kTt[p0:p0 + DH, pr, bass.ts(m, 128)],
                                qT[p0:p0 + DH, pr, :],
                                start=True, stop=True, tile_position=tp)
                            e_t = sb.tile([128, S], ADT, tag="exp",
                                          name="expt", bufs=16)
                            nc.scalar.activation(e_t[:], s_ps[:], AF.Exp,
                                                 bias=ext[:, m:m + 1],
                                                 scale=SCALE)
                            exps[(hh, m)] = e_t
                    return exps

                def pv(ps, pr, exps):
                    # Both heads' PV into one 2-bank PSUM tile so the two
                    # softmax denominator rows (partition 64) sit in one
                    # contiguous free range: one DVE copy to SBUF, then the
                    # custom-DVE fast reciprocal (bf16 out). The custom op
                    # reads garbage from PSUM on real HW (SBUF-only), hence
                    # the copy.
                    c2 = ps.tile([128, 2, S], F32, tag="ctx", name="cps",
                                 bufs=2)
                    for hh in range(2):
                        h = 2 * pr + hh
                        for m in range(ST):
                            nc.tensor.matmul(c2[0:DH + 1, hh, :],
                                             v_aug[:, m, h, :],
                                             exps[(hh, m)][:],
                                             start=(m == 0), stop=(m == ST - 1))
                    den = sb.tile([1, 2, S], F32, tag="den", name="den",
                                  bufs=3)
                    nc.vector.tensor_copy(den[:], c2[DH:DH + 1, :, :])
                    rcp2 = sb.tile([1, 2, S], ADT, tag="rcp2", name="rcp2",
                                   bufs=3)
                    c = RECIP_APPROX_FAST_CONSTS
                    nc.vector._custom_dve(
                        RECIPROCAL_APPROX_FAST,
                        out=rcp2[:].rearrange("p a s -> p (a s)"),
                        in0=den[:].rearrange("p a s -> p (a s)"),
                        s0=c["s0"], s1=c["s1"], imm2=c["imm2"])
                    return (pr, c2, rcp2)

                def att_finalize(ps, pend):
                    # bc_ps shares the scores pool rotation (no extra bank)
                    pr_, c2_, rcp2_ = pend
                    bc_ps = ps.tile([128, S], F32, tag="scores", name="bcps",
                                    bufs=2)
                    for hh in range(2):
                        nc.tensor.matmul(bc_ps[hh * DH:(hh + 1) * DH, :],
                                         ones64[:], rcp2_[:, hh, :],
                                         start=True, stop=True)
                    bc_sb = sb.tile([128, S], ADT, tag="bcsb", name="bcsb",
                                    bufs=2)
                    nc.vector.tensor_copy(bc_sb[:], bc_ps[:])
                    for hh in range(2):
                        p0 = hh * DH
                        nc.vector.tensor_mul(
                            qT[p0:p0 + DH, pr_, :],
                            c2_[0:DH, hh, :],
                            bc_sb[p0:p0 + DH, :])

                with tc.tile_pool(name="ps_qk", bufs=1, space="PSUM") as psqk:
                    # pair 0 projections + scores, then V while its exps run
                    proj_qk(psqk, 0)
                    with tc.tile_pool(name="ps_sc", bufs=1,
                                      space="PSUM") as pssc:
                        exps0 = scores_exp(pssc, 0)
                        # V projection (seq-major, bias via K=1 ones matmul)
                        bv_row = sb.tile([1, H], ADT, tag="bvrow",
                                         name="bvrow", bufs=2)
                        nc.sync.dma_start(out=bv_row[:], in_=BVB.ap()[l:l + 1, :])
                        with tc.tile_pool(name="ps_v", bufs=1,
                                          space="PSUM") as psv:
                            for s in range(ST):
                                p_a = psv.tile([128, S], F32, tag="va",
                                               name="pva", bufs=2)
                                p_b = psv.tile([128, 256], F32, tag="vb",
                                               name="pvb", bufs=2)
                                for k in range(KT):
                                    nc.tensor.matmul(
                                        p_a[:], xT[:, k, bass.ts(s, 128)],
                                        wv[k][:, 0:512],
                                        start=(k == 0), stop=False)
                                    nc.tensor.matmul(
                                        p_b[:], xT[:, k, bass.ts(s, 128)],
                                        wv[k][:, 512:768],
                                        start=(k == 0), stop=False)
                                nc.tensor.matmul(p_a[:], ones128pc[:],
                                                 bv_row[:, 0:512],
                                                 start=False, stop=True)
                                nc.tensor.matmul(p_b[:], ones128pc[:],
                                                 bv_row[:, 512:768],
                                                 start=False, stop=True)
                                nc.vector.tensor_copy(
                                    v_aug[:, s, 0:8, 0:DH],
                                    p_a[:].rearrange("p (h c) -> p h c", c=DH))
                                nc.vector.tensor_copy(
                                    v_aug[:, s, 8:12, 0:DH],
                                    p_b[:].rearrange("p (h c) -> p h c", c=DH))
                        # steady pair pipeline
                        with tc.tile_pool(name="ps_ctx", bufs=1,
                                          space="PSUM") as psctx:
                            pending = None
                            exps = exps0
                            for pr in range(NP):
                                if pr + 1 < NP:
                                    proj_qk(psqk, pr + 1)
                                # finalize pair pr-1 now: its reciprocal was
                                # issued a full pair-iteration ago, so the
                                # broadcast matmuls at the PE queue head
                                # never wait on the DVE chain — and its
                                # scores-pool slot was drained by exps(pr)
                                # long ago.
                                if pending is not None:
                                    att_finalize(pssc, pending)
                                nxt = (scores_exp(pssc, pr + 1)
                                       if pr + 1 < NP else None)
                                pending = pv(psctx, pr, exps)
                                exps = nxt
                            att_finalize(pssc, pending)

                if KDBG and l == 0:
                    dq = pers.tile([128, KT, S], F32, tag="dbgq")
                    nc.vector.tensor_copy(dq[:], qT[:])
                    nc.sync.dma_start(
                        out=DQ.ap().rearrange("(k p) s -> p k s", p=128),
                        in_=dq[:])
                    dk = pers.tile([128, KT, S], F32, tag="dbgk")
                    nc.vector.tensor_copy(dk[:], kTt[:])
                    nc.sync.dma_start(
                        out=DK.ap().rearrange("(k p) s -> p k s", p=128),
                        in_=dk[:])

                # ================= Wo + residual + LN1 =================
                with tc.tile_pool(name="ps_wo", bufs=1, space="PSUM") as ps:
                    wo = [load_w768(WO, l, k, f"wo{k}") for k in range(KT)]
                    st_ps = ps.tile([1, 2, S], F32, tag="sum", name="sum")
                    for m in range(MT):
                        p_o = ps.tile([128, S], F32, tag="proj", name="po",
                                      bufs=3)
                        for k in range(KT):
                            nc.tensor.matmul(p_o[:], wo[k][:, bass.ts(m, 128)],
                                             qT[:, k, :], start=(k == 0),
                                             stop=(k == KT - 1))
                        # y = (psum + bo) + x   (into kTt, reused as y)
                        nc.vector.scalar_tensor_tensor(
                            kTt[:, m, :], p_o[:], bo_t[:, l, m:m + 1],
                            xT[:, m, :], op0=ALU.add, op1=ALU.add)
                        ln_sums(ps, kTt, m, m == 0, m == MT - 1, st_ps)
                    ln_finalize(ps, kTt, g1_t, b1_t, l, attnT, st_ps, warm=6)

                # ================= FFN =================
                with tc.tile_pool(name="ps_ffn", bufs=1, space="PSUM") as ps:
                    ffo = [ps.tile([128, S], F32, tag="ffo", name=f"ffo{m}",
                                   bufs=6)
                           for m in range(MT)]
                    # software-pipelined: p_f chain of ko+1 issues before the
                    # ffo accumulation of ko so the PE isn't starved while the
                    # gelu of ko drains.
                    ffts = {}
                    pf01 = []

                    def ffn1_weights(ko):
                        wi_t = wf1pool.tile([128, KT, 128], WDT, tag="wff1",
                                            name=f"wi{ko}")
                        nc.sync.dma_start(out=wi_t[:], in_=WI.ap()[l, ko])
                        wf_t = wpool.tile([128, H], WDT, tag="w768",
                                          name=f"wf{ko}")
                        nc.sync.dma_start(out=wf_t[:],
                                          in_=WF.ap()[l, bass.ts(ko, 128), :])
                        return wi_t, wf_t

                    def ffn1_finish(ko, wi_t, wf_t, p_f):
                        ff_t = sb.tile([128, S], ADT, tag="fft", name="fft",
                                       bufs=4)
                        nc.scalar.activation(ff_t[:], p_f[:], AF.Gelu,
                                             bias=bi_t[:, l, ko:ko + 1])
                        ffts[ko] = (ff_t, wf_t)

                    for ko in range(IT + 1):
                        if ko < IT:
                            wi_t, wf_t = ffn1_weights(ko)
                            p_f = ps.tile([128, S], F32, tag="ff1", name="pf",
                                          bufs=2)
                            for k in range(KT):
                                nc.tensor.matmul(p_f[:], wi_t[:, k, :],
                                                 attnT[:, k, :],
                                                 start=(k == 0),
                                                 stop=(k == KT - 1))
                            ffn1_finish(ko, wi_t, wf_t, p_f)
                        if ko >= 1:
                            ff_p, wf_p = ffts.pop(ko - 1)
                            for m in range(MT):
                                nc.tensor.matmul(ffo[m][:],
                                                 wf_p[:, bass.ts(m, 128)],
                                                 ff_p[:], start=(ko - 1 == 0),
                                                 stop=(ko - 1 == IT - 1))
                    # dummy 1-elem ln: hoists the natural_log_exp-set
                    # ACT_TABLE_LOAD off LN2's serial chain (Act is idle
                    # after the last gelu while FFN2 drains).
                    scr2 = sb.tile([1, 1], F32, tag="scr", name="scr2", bufs=2)
                    nc.scalar.activation(scr2[:], eps_t[:], AF.Ln)
                    for m in range(MT):
                        # y2 = (ffo + bf) + attnT   (into kTt)
                        nc.vector.scalar_tensor_tensor(
                            kTt[:, m, :], ffo[m][:], bf_t[:, l, m:m + 1],
                            attnT[:, m, :], op0=ALU.add, op1=ALU.add)
                with tc.tile_pool(name="ps_ln2", bufs=1, space="PSUM") as ps:
                    st_ps = ps.tile([1, 2, S], F32, tag="sum", name="sum")
                    ln_sums_split(ps, kTt, st_ps)
                    # On the last pass the LN2 normalize writes the fp32
                    # output tile directly (skips a 3us full-width copy).
                    ln_finalize(ps, kTt, g2_t, b2_t, l,
                                xout if last else xT, st_ps, warm=6)

            xout = pers.tile([128, KT, S], F32, tag="xout")
            for r in range(repeat):
                for l in range(n_layers):
                    layer_body(l, last=(r == repeat - 1 and l == n_layers - 1))

            nc.sync.dma_start(
                out=OUT.ap().rearrange("(k p) s -> p k s", p=128),
                in_=xout[:])

    nc.compile()
    return nc


_CACHE = {}


def get_program(repeat=1, n_layers=L):
    key = (repeat, n_layers)
    if key not in _CACHE:
        _CACHE[key] = build_program(repeat, n_layers)
    return _CACHE[key]


def make_input_maps(inputs):
    """Per-core input maps from the full-batch input dict."""
    import ml_dtypes
    wnp = ml_dtypes.bfloat16 if WDT == BF16 else np.float32
    anp = ml_dtypes.bfloat16 if ADT == BF16 else np.float32
    hs = np.ascontiguousarray(np.asarray(inputs["hidden_states"], np.float32))
    mask = np.asarray(inputs["attention_mask"], np.float32)
    wi = np.ascontiguousarray(
        np.asarray(inputs["Wi"], np.float32).reshape(L, KT, 128, IT, 128)
        .transpose(0, 3, 2, 1, 4)).astype(wnp)
    shared = {
        "WQ": np.ascontiguousarray(np.asarray(inputs["Wq"], np.float32)).astype(wnp),
        "WK": np.ascontiguousarray(np.asarray(inputs["Wk"], np.float32)).astype(wnp),
        "WV": np.ascontiguousarray(np.asarray(inputs["Wv"], np.float32)).astype(wnp),
        "WO": np.ascontiguousarray(np.asarray(inputs["Wo"], np.float32)).astype(wnp),
        "WI": wi,
        "WF": np.ascontiguousarray(np.asarray(inputs["Wf"], np.float32)).astype(wnp),
        "BVB": np.asarray(inputs["bv"], np.float32).astype(anp),
        "PP8": np.ascontiguousarray(np.stack(
            [np.asarray(inputs[k], np.float32).reshape(L, KT, 128)
             .transpose(2, 0, 1)
             for k in ("bq", "bk", "bo", "bf", "ln1_g", "ln1_b",
                       "ln2_g", "ln2_b")], axis=1)),
        "PBI": np.ascontiguousarray(
            np.asarray(inputs["bi"], np.float32).reshape(L, IT, 128)
            .transpose(2, 0, 1)),
    }
    in_maps = []
    for c in range(B):
        ext = ((1.0 - mask[c]) * -10000.0).astype(np.float32).reshape(ST, 128)
        in_maps.append({
            "XT": np.ascontiguousarray(hs[c].T).astype(anp),
            "EXTM": ext,
            **shared,
        })
    return in_maps


def kernel(**inputs):
    nc = get_program(repeat=1)
    in_maps = make_input_maps(inputs)
    res = run_bass_kernel_spmd(nc, in_maps, list(range(B)))
    out = np.stack([res.results[c]["OUT"].T for c in range(B)], axis=0)
    return out.astype(np.float32)


# revision 67
# speedup vs baseline: 1489.3405x; 1.0024x over previous
"""BERT-base 12-layer encoder forward on 8 trn2 NeuronCores.

Strategy: pure data parallelism — batch B=8, one sequence per core, full
weights replicated (bf16 in HBM, halving weight DMA), zero collectives.
All matmul operands (weights AND activations) are bf16 — the PE runs
bf16 at the same 1 column/cycle as fp32r, so this costs nothing on the
tensor engine while halving DMA and SBUF traffic; accumulation, LN
statistics, and softmax denominators stay fp32 in PSUM. Activations are
feature-major ([hidden, seq], hidden on SBUF partitions) so every
projection is a PE matmul with the weight stationary.

v2 changes vs baseline:
- QKV projections for head-pair p+1 are interleaved with the
  scores/exp/PV of pair p, so the Act engine's softmax-exp stream
  (~27us/layer, the attention bottleneck) hides behind the PE's
  projection matmuls instead of serializing after them.
- Softmax 1/denom uses reciprocal_approx_fast (1 DVE pass, ~18-bit)
  instead of the iterative-divide reciprocal (8 cyc/elem on a single
  partition lane).
- LayerNorm rstd = exp(-0.5*ln(var+eps)) on the Act engine: ln and exp
  live in the same activation-table set as the softmax exp
  (natural_log_exp_and_others), so the per-layer sqrt-set switches
  (~2.7us each, 2/layer) and the slow DVE reciprocal disappear.
"""
import sys

sys.path.insert(0, "/opt/trn_rl_repo")

import numpy as np
import concourse.bass as bass
import concourse.mybir as mybir
import concourse.tile as tile
from concourse import bacc
from concourse.bass_utils import run_bass_kernel_spmd
from concourse.dve_ops import RECIPROCAL_APPROX_FAST, RECIP_APPROX_FAST_CONSTS

F32 = mybir.dt.float32
F32R = mybir.dt.float32r
BF16 = mybir.dt.bfloat16
AF = mybir.ActivationFunctionType
ALU = mybir.AluOpType

L, H, NH, I = 12, 768, 12, 3072
DH = 64
B, S = 8, 512
KT = H // 128          # 6 k-tiles over hidden
MT = H // 128          # 6 m-tiles over hidden
IT = I // 128          # 24 tiles over intermediate
ST = S // 128          # 4 tiles over sequence
NP = NH // 2           # 6 head pairs
EPS = 1e-12
SCALE = 0.125          # 1/sqrt(64)
RSQH = 1.0 / float(np.sqrt(H))
WDT = BF16             # weight dtype in HBM
ADT = BF16             # activation dtype (matmul operands)


def build_program(repeat=1, n_layers=L):
    # Build-time activation-table steering: the default greedy set picker
    # chooses exp_and_others for Exp and natural_log for Ln, costing 6
    # ACT_TABLE_LOADs (~2.7us each on HW) per layer. Restricting the
    # visible sets to natural_log_exp_and_others (exp+ln+square+identity)
    # and gelu_and_others makes every layer need exactly 2 loads
    # (exp-set <-> gelu-set around the FFN). Indices are preserved, so the
    # emitted act_func_set_id still refers to the real act_info.json
    # entries. Restored right after the build.
    import concourse.hw_specs as hw_specs
    _orig_gat = hw_specs.get_activation_tables
    _keep = {"natural_log_exp_and_others", "gelu_and_others"}

    def _patched_gat(module_arch):
        tabs = _orig_gat(module_arch)
        return {name: (fns if name in _keep else set())
                for name, fns in tabs.items()}

    hw_specs.get_activation_tables = _patched_gat
    import concourse.bacc as bacc_mod
    _bacc_had = getattr(bacc_mod, "get_activation_tables", None)
    if _bacc_had is not None:
        bacc_mod.get_activation_tables = _patched_gat
    try:
        return _build_program_inner(repeat, n_layers)
    finally:
        hw_specs.get_activation_tables = _orig_gat
        if _bacc_had is not None:
            bacc_mod.get_activation_tables = _bacc_had


def _build_program_inner(repeat=1, n_layers=L):
    nc = bacc.Bacc("TRN2", target_bir_lowering=False)

    XT = nc.dram_tensor("XT", [H, S], ADT, kind="ExternalInput")
    EXTM = nc.dram_tensor("EXTM", [ST, 128], F32, kind="ExternalInput")
    WQ = nc.dram_tensor("WQ", [L, H, H], WDT, kind="ExternalInput")
    WK = nc.dram_tensor("WK", [L, H, H], WDT, kind="ExternalInput")
    WV = nc.dram_tensor("WV", [L, H, H], WDT, kind="ExternalInput")
    WO = nc.dram_tensor("WO", [L, H, H], WDT, kind="ExternalInput")
    WI = nc.dram_tensor("WI", [L, IT, 128, KT, 128], WDT, kind="ExternalInput")
    WF = nc.dram_tensor("WF", [L, I, H], WDT, kind="ExternalInput")
    BVB = nc.dram_tensor("BVB", [L, H], ADT, kind="ExternalInput")
    # 8 per-layer param vectors pre-transposed host-side to the on-chip
    # feature-major layout [128, L, KT] and packed into one contiguous
    # tensor (one big DMA instead of 9 scatter-pattern DMAs with ~72
    # descriptors per partition each).
    PP8 = nc.dram_tensor("PP8", [128, 8, L, KT], F32, kind="ExternalInput")
    PBI = nc.dram_tensor("PBI", [128, L, IT], F32, kind="ExternalInput")
    OUT = nc.dram_tensor("OUT", [H, S], F32, kind="ExternalOutput")
    import os
    KDBG = os.environ.get("KDBG") == "1"
    if KDBG:
        DQ = nc.dram_tensor("DQ", [H, S], F32, kind="ExternalOutput")
        DK = nc.dram_tensor("DK", [H, S], F32, kind="ExternalOutput")

    with tile.TileContext(nc) as tc:
        with (
            nc.allow_low_precision(reason="bf16 matmul pipeline"),
            tc.tile_pool(name="pers", bufs=1) as pers,
            tc.tile_pool(name="w768", bufs=26) as wpool,
            tc.tile_pool(name="wff1", bufs=4) as wf1pool,
            tc.tile_pool(name="sb", bufs=2) as sb,
        ):
            # ---- persistent activations ----
            xT = pers.tile([128, KT, S], ADT, tag="xT")
            nc.sync.dma_start(out=xT[:], in_=XT.ap().rearrange(
                "(k p) s -> p k s", p=128))
            qT = pers.tile([128, KT, S], ADT, tag="qT")    # reused as ctxT
            kTt = pers.tile([128, KT, S], ADT, tag="kTt")  # reused as LN input y
            attnT = pers.tile([128, KT, S], ADT, tag="attnT")
            v_aug = pers.tile([128, ST, NH, DH + 1], ADT, tag="vaug")
            nc.vector.memset(v_aug[:, :, :, DH], 1.0)

            ext = pers.tile([128, ST], F32, tag="ext")
            nc.sync.dma_start(out=ext[:], in_=EXTM.ap().rearrange("k p -> p k"))

            # ---- constants ----
            ones128c = pers.tile([1, 128], ADT, tag="ones128c")
            nc.vector.memset(ones128c[:], 1.0)
            invh128c = pers.tile([1, 128], ADT, tag="invh128c")
            nc.vector.memset(invh128c[:], 1.0 / H)
            ones128p = pers.tile([128, 1], ADT, tag="ones128p")
            nc.vector.memset(ones128p[:], 1.0)
            ones128pc = pers.tile([1, 128], ADT, tag="ones128pc")
            nc.vector.memset(ones128pc[:], 1.0)
            ones64 = pers.tile([1, DH], ADT, tag="ones64")
            nc.vector.memset(ones64[:], 1.0)
            eps_t = pers.tile([1, 1], F32, tag="eps")
            nc.vector.memset(eps_t[:], EPS)

            # ---- per-layer params, loaded once (feature-major [128, L, KT],
            # pre-transposed host-side; single contiguous DMA) ----
            pp8 = pers.tile([128, 8, L, KT], F32, tag="pp8")
            nc.sync.dma_start(out=pp8[:], in_=PP8.ap())
            bi_t = pers.tile([128, L, IT], F32, tag="pbi")
            nc.sync.dma_start(out=bi_t[:], in_=PBI.ap())
            bq_t = pp8[:, 0]; bk_t = pp8[:, 1]
            bo_t = pp8[:, 2]; bf_t = pp8[:, 3]
            g1_t = pp8[:, 4]; b1_t = pp8[:, 5]
            g2_t = pp8[:, 6]; b2_t = pp8[:, 7]

            def ln_sums(ps, y, k, first, last, st_ps):
                """Accumulate sum / sumsq of y k-tile into st_ps rows.
                Squares ride the idle GPSIMD engine so the Act engine's
                queue stays clear for the stats chain that follows."""
                nc.tensor.matmul(st_ps[:, 0, :], ones128p[:], y[:, k, :],
                                 start=first, stop=last)
                sq = sb.tile([128, S], ADT, tag="sq", name="sq", bufs=4)
                nc.gpsimd.tensor_mul(sq[:], y[:, k, :], y[:, k, :])
                nc.tensor.matmul(st_ps[:, 1, :], ones128p[:], sq[:],
                                 start=first, stop=last)

            def ln_sums_split(ps, y, st_ps):
                """Sums in separate pipelined loops (for a phase tail where
                the PE has no other work to hide per-tile latencies).
                Squares alternate gpsimd/Act: the Act engine is idle after
                the last gelu, and gpsimd alone was the ~6.4us long pole."""
                for k in range(KT):
                    nc.tensor.matmul(st_ps[:, 0, :], ones128p[:], y[:, k, :],
                                     start=(k == 0), stop=(k == KT - 1))
                for k in range(KT):
                    sq = sb.tile([128, S], ADT, tag="sq", name="sq", bufs=4)
                    if k % 2 == 0:
                        nc.gpsimd.tensor_mul(sq[:], y[:, k, :], y[:, k, :])
                    else:
                        nc.scalar.activation(sq[:], y[:, k, :], AF.Square)
                    nc.tensor.matmul(st_ps[:, 1, :], ones128p[:], sq[:],
                                     start=(k == 0), stop=(k == KT - 1))

            def ln_finalize(ps, y, gam, bet, l, out, st_ps, warm=0,
                            preload_gelu=False):
                """LN stats + normalize.

                rstd = exp(-0.5*ln(varh/H + eps)) keeps the whole chain in
                the natural_log_exp activation-table set (no sqrt-set
                switch, no slow DVE reciprocal).

                `warm` > 0 issues dummy ones-matmuls into a scratch PSUM
                bank while the stats chain runs — the PE p-state drops to
                1.2GHz after a ~3.4us idle gap, so keeping it streaming
                through this serial chain makes the next phase's matmuls
                start at full clock."""
                if warm:
                    w_ps = ps.tile([1, S], F32, tag="warm", name="warm")
                    for w in range(warm):
                        nc.tensor.matmul(w_ps[:], ones128p[:], y[:, w % KT, :],
                                         start=True, stop=True)
                sums = sb.tile([1, S], ADT, tag="sums", name="sums", bufs=2)
                nc.vector.tensor_copy(sums[:], st_ps[:, 0, :])
                # s2h = (sum/sqrt(H))^2 straight from PSUM on Act
                s2h = sb.tile([1, S], F32, tag="s2h", name="s2h", bufs=2)
                nc.scalar.activation(s2h[:], st_ps[:, 0, :], AF.Square,
                                     scale=RSQH)
                varh = sb.tile([1, S], F32, tag="varh", name="varh", bufs=2)
                nc.vector.tensor_sub(varh[:], st_ps[:, 1, :], s2h[:])
                lnv = sb.tile([1, S], F32, tag="lnv", name="lnv", bufs=2)
                nc.scalar.activation(lnv[:], varh[:], AF.Ln,
                                     bias=eps_t[:], scale=1.0 / H)
                rstd = sb.tile([1, S], ADT, tag="rstd", name="rstd", bufs=2)
                nc.scalar.activation(rstd[:], lnv[:], AF.Exp, scale=-0.5)
                mub_ps = ps.tile([128, S], F32, tag="mub", name="mub", bufs=1)
                nc.tensor.matmul(mub_ps[:], invh128c[:], sums[:],
                                 start=True, stop=True)
                rsb_ps = ps.tile([128, S], F32, tag="rsb", name="rsb", bufs=1)
                nc.tensor.matmul(rsb_ps[:], ones128c[:], rstd[:],
                                 start=True, stop=True)
                if warm:
                    for w in range(warm):
                        nc.tensor.matmul(w_ps[:], ones128p[:], y[:, w % KT, :],
                                         start=True, stop=True)
                # bf16 SBUF copies of the broadcasts: the GPSIMD half needs
                # SBUF (no PSUM port), and bf16 operands put the DVE
                # tensor_tensor ops in 2x mode (424ns vs 690ns a tile).
                # Mean/rstd in bf16 cost ~1e-3 relative — well inside
                # tolerance. DVE tile 0 reads PSUM directly so the first
                # output tile doesn't wait for the copies.
                mub_sb = sb.tile([128, S], ADT, tag="mubsb", name="mubsb", bufs=2)
                nc.scalar.copy(mub_sb[:], mub_ps[:])
                rsb_sb = sb.tile([128, S], ADT, tag="rsbsb", name="rsbsb", bufs=2)
                nc.scalar.copy(rsb_sb[:], rsb_ps[:])
                for k in range(KT):
                    dve = k % 2 == 0
                    eng = nc.vector if dve else nc.gpsimd
                    first = dve and k == 0
                    mu_in = mub_ps[:] if first else mub_sb[:]
                    rs_in = rsb_ps[:] if first else rsb_sb[:]
                    t1 = sb.tile([128, S], F32 if first else ADT, tag="lnt1",
                                 name="lnt1", bufs=4)
                    eng.tensor_sub(t1[:], y[:, k, :], mu_in)
                    t2 = sb.tile([128, S], ADT, tag="lnt2", name="lnt2", bufs=4)
                    eng.tensor_mul(t2[:], t1[:], rs_in)
                    nc.scalar.activation(out[:, k, :], t2[:], AF.Identity,
                                         bias=bet[:, l, k:k + 1],
                                         scale=gam[:, l, k:k + 1])
                    if preload_gelu and k == 0:
                        # dummy 1-elem gelu right after the first normalize
                        # identity: the gelu-set ACT_TABLE_LOAD lands in the
                        # Act engine's slack between identity tiles (paced
                        # by the DVE/gpsimd normalize) instead of on the
                        # FFN critical path. Later identities exist in the
                        # gelu set too, so no reload.
                        scr = sb.tile([1, 1], F32, tag="scr", name="scr",
                                      bufs=2)
                        nc.scalar.activation(scr[:], eps_t[:], AF.Gelu)

            def load_w768(dram, l, k, name):
                w = wpool.tile([128, H], WDT, tag="w768", name=name)
                nc.sync.dma_start(out=w[:], in_=dram.ap()[l, bass.ts(k, 128), :])
                return w

            def layer_body(l, last=False):
                # ============ QKV + attention, pair-pipelined ============
                # Q is projected first in two k-outer groups of 3 m-tiles
                # (3 PSUM banks): the k-outer order lets each matmul start
                # as soon as the previous layer's LN2 normalize emits that
                # xT k-tile, filling the LN2 chain's PE-idle window.
                # Head-pair m then needs exactly the m-th K-projection
                # tile, so K is computed per-pair inside the softmax
                # pipeline: while the Act engine streams exp tiles for
                # pair p+1, the PE runs K_{p+1} and PV_p.
                wq = [load_w768(WQ, l, k, f"wq{k}") for k in range(KT)]
                wk = [load_w768(WK, l, k, f"wk{k}") for k in range(KT)]
                wv = [load_w768(WV, l, k, f"wv{k}") for k in range(KT)]

                def proj_qk(ps, m):
                    p_q = ps.tile([128, S], F32, tag="qk", name="pq", bufs=2)
                    for k in range(KT):
                        nc.tensor.matmul(p_q[:], wq[k][:, bass.ts(m, 128)],
                                         xT[:, k, :], start=(k == 0),
                                         stop=(k == KT - 1))
                    nc.scalar.activation(qT[:, m, :], p_q[:], AF.Identity,
                                         bias=bq_t[:, l, m:m + 1])
                    p_k = ps.tile([128, S], F32, tag="qk", name="pk", bufs=2)
                    for k in range(KT):
                        nc.tensor.matmul(p_k[:], wk[k][:, bass.ts(m, 128)],
                                         xT[:, k, :], start=(k == 0),
                                         stop=(k == KT - 1))
                    nc.scalar.activation(kTt[:, m, :], p_k[:], AF.Identity,
                                         bias=bk_t[:, l, m:m + 1])

                def scores_exp(ps, pr):
                    exps = {}
                    for hh in range(2):
                        p0 = hh * DH
                        tp = None if hh == 0 else (64, 0)
                        for m in range(ST):
                            s_ps = ps.tile([128, S], F32, tag="scores",
                                           name="sps", bufs=2)
                            nc.tensor.matmul(
                                s_ps[:],
                                kTt[p0:p0 + DH, pr, bass.ts(m, 128)],
                                qT[p0:p0 + DH, pr, :],
                                start=True, stop=True, tile_position=tp)
                            e_t = sb.tile([128, S], ADT, tag="exp",
                                          name="expt", bufs=16)
                            nc.scalar.activation(e_t[:], s_ps[:], AF.Exp,
                                                 bias=ext[:, m:m + 1],
                                                 scale=SCALE)
                            exps[(hh, m)] = e_t
                    return exps

                def pv_split(ps, pr, exps):
                    # Last-pair variant: per-head denominator copy/recip so
                    # head 0's chain runs while head 1's PV matmuls are
                    # still accumulating — shortens the attention tail the
                    # Wo phase waits on.
                    c2 = ps.tile([128, 2, S], F32, tag="ctx", name="cps",
                                 bufs=2)
                    c = RECIP_APPROX_FAST_CONSTS
                    rcp2 = sb.tile([1, 2, S], ADT, tag="rcp2", name="rcp2",
                                   bufs=3)
                    for hh in range(2):
                        h = 2 * pr + hh
                        for m in range(ST):
                            nc.tensor.matmul(c2[0:DH + 1, hh, :],
                                             v_aug[:, m, h, :],
                                             exps[(hh, m)][:],
                                             start=(m == 0), stop=(m == ST - 1))
                        den = sb.tile([1, S], F32, tag="den1", name="den1",
                                      bufs=2)
                        nc.vector.tensor_copy(den[:], c2[DH:DH + 1, hh, :])
                        nc.vector._custom_dve(
                            RECIPROCAL_APPROX_FAST, out=rcp2[:, hh, :],
                            in0=den[:], s0=c["s0"], s1=c["s1"],
                            imm2=c["imm2"])
                    return (pr, c2, rcp2)

                def pv(ps, pr, exps):
                    # Both heads' PV into one 2-bank PSUM tile so the two
                    # softmax denominator rows (partition 64) sit in one
                    # contiguous free range: one DVE copy to SBUF, then the
                    # custom-DVE fast reciprocal (bf16 out). The custom op
                    # reads garbage from PSUM on real HW (SBUF-only), hence
                    # the copy.
                    c2 = ps.tile([128, 2, S], F32, tag="ctx", name="cps",
                                 bufs=2)
                    for hh in range(2):
                        h = 2 * pr + hh
                        for m in range(ST):
                            nc.tensor.matmul(c2[0:DH + 1, hh, :],
                                             v_aug[:, m, h, :],
                                             exps[(hh, m)][:],
                                             start=(m == 0), stop=(m == ST - 1))
                    den = sb.tile([1, 2, S], F32, tag="den", name="den",
                                  bufs=3)
                    nc.vector.tensor_copy(den[:], c2[DH:DH + 1, :, :])
                    rcp2 = sb.tile([1, 2, S], ADT, tag="rcp2", name="rcp2",
                                   bufs=3)
                    c = RECIP_APPROX_FAST_CONSTS
                    nc.vector._custom_dve(
                        RECIPROCAL_APPROX_FAST,
                        out=rcp2[:].rearrange("p a s -> p (a s)"),
                        in0=den[:].rearrange("p a s -> p (a s)"),
                        s0=c["s0"], s1=c["s1"], imm2=c["imm2"])
                    return (pr, c2, rcp2)

                def att_finalize(ps, pend):
                    # bc_ps shares the scores pool rotation (no extra bank)
                    pr_, c2_, rcp2_ = pend
                    bc_ps = ps.tile([128, S], F32, tag="scores", name="bcps",
                                    bufs=2)
                    for hh in range(2):
                        nc.tensor.matmul(bc_ps[hh * DH:(hh + 1) * DH, :],
                                         ones64[:], rcp2_[:, hh, :],
                                         start=True, stop=True)
                    bc_sb = sb.tile([128, S], ADT, tag="bcsb", name="bcsb",
                                    bufs=2)
                    nc.vector.tensor_copy(bc_sb[:], bc_ps[:])
                    for hh in range(2):
                        p0 = hh * DH
                        nc.vector.tensor_mul(
                            qT[p0:p0 + DH, pr_, :],
                            c2_[0:DH, hh, :],
                            bc_sb[p0:p0 + DH, :])

                with tc.tile_pool(name="ps_qk", bufs=1, space="PSUM") as psqk:
                    # pair 0 projections + scores, then V while its exps run
                    proj_qk(psqk, 0)
                    with tc.tile_pool(name="ps_sc", bufs=1,
                                      space="PSUM") as pssc:
                        exps0 = scores_exp(pssc, 0)
                        # V projection (seq-major, bias via K=1 ones matmul)
                        bv_row = sb.tile([1, H], ADT, tag="bvrow",
                                         name="bvrow", bufs=2)
                        nc.sync.dma_start(out=bv_row[:], in_=BVB.ap()[l:l + 1, :])
                        with tc.tile_pool(name="ps_v", bufs=1,
                                          space="PSUM") as psv:
                            for s in range(ST):
                                p_a = psv.tile([128, S], F32, tag="va",
                                               name="pva", bufs=2)
                                p_b = psv.tile([128, 256], F32, tag="vb",
                                               name="pvb", bufs=2)
                                for k in range(KT):
                                    nc.tensor.matmul(
                                        p_a[:], xT[:, k, bass.ts(s, 128)],
                                        wv[k][:, 0:512],
                                        start=(k == 0), stop=False)
                                    nc.tensor.matmul(
                                        p_b[:], xT[:, k, bass.ts(s, 128)],
                                        wv[k][:, 512:768],
                                        start=(k == 0), stop=False)
                                nc.tensor.matmul(p_a[:], ones128pc[:],
                                                 bv_row[:, 0:512],
                                                 start=False, stop=True)
                                nc.tensor.matmul(p_b[:], ones128pc[:],
                                                 bv_row[:, 512:768],
                                                 start=False, stop=True)
                                nc.vector.tensor_copy(
                                    v_aug[:, s, 0:8, 0:DH],
                                    p_a[:].rearrange("p (h c) -> p h c", c=DH))
                                nc.vector.tensor_copy(
                                    v_aug[:, s, 8:12, 0:DH],
                                    p_b[:].rearrange("p (h c) -> p h c", c=DH))
                        # steady pair pipeline
                        with tc.tile_pool(name="ps_ctx", bufs=1,
                                          space="PSUM") as psctx:
                            pending = None
                            exps = exps0
                            for pr in range(NP):
                                if pr + 1 < NP:
                                    proj_qk(psqk, pr + 1)
                                # finalize pair pr-1 now: its reciprocal was
                                # issued a full pair-iteration ago, so the
                                # broadcast matmuls at the PE queue head
                                # never wait on the DVE chain — and its
                                # scores-pool slot was drained by exps(pr)
                                # long ago.
                                if pending is not None:
                                    att_finalize(pssc, pending)
                                nxt = (scores_exp(pssc, pr + 1)
                                       if pr + 1 < NP else None)
                                pending = (pv(psctx, pr, exps)
                                           if pr + 1 < NP
                                           else pv_split(psctx, pr, exps))
                                exps = nxt
                            # last pair: per-head broadcast/normalize so
                            # head 0's qT tile lands while head 1's chain
                            # still runs (Wo waits on this tail).
                            pr_, c2_, rcp2_ = pending
                            bc_ps = pssc.tile([128, S], F32, tag="scores",
                                              name="bcps", bufs=2)
                            bc_sb = sb.tile([128, S], ADT, tag="bcsb",
                                            name="bcsb", bufs=2)
                            for hh in range(2):
                                p0 = hh * DH
                                nc.tensor.matmul(bc_ps[p0:p0 + DH, :],
                                                 ones64[:], rcp2_[:, hh, :],
                                                 start=True, stop=True)
                                nc.vector.tensor_copy(bc_sb[p0:p0 + DH, :],
                                                      bc_ps[p0:p0 + DH, :])
                                nc.vector.tensor_mul(
                                    qT[p0:p0 + DH, pr_, :],
                                    c2_[0:DH, hh, :],
                                    bc_sb[p0:p0 + DH, :])

                if KDBG and l == 0:
                    dq = pers.tile([128, KT, S], F32, tag="dbgq")
                    nc.vector.tensor_copy(dq[:], qT[:])
                    nc.sync.dma_start(
                        out=DQ.ap().rearrange("(k p) s -> p k s", p=128),
                        in_=dq[:])
                    dk = pers.tile([128, KT, S], F32, tag="dbgk")
                    nc.vector.tensor_copy(dk[:], kTt[:])
                    nc.sync.dma_start(
                        out=DK.ap().rearrange("(k p) s -> p k s", p=128),
                        in_=dk[:])

                # ================= Wo + residual + LN1 =================
                with tc.tile_pool(name="ps_wo", bufs=1, space="PSUM") as ps:
                    wo = [load_w768(WO, l, k, f"wo{k}") for k in range(KT)]
                    st_ps = ps.tile([1, 2, S], F32, tag="sum", name="sum")
                    for m in range(MT):
                        p_o = ps.tile([128, S], F32, tag="proj", name="po",
                                      bufs=3)
                        for k in range(KT):
                            nc.tensor.matmul(p_o[:], wo[k][:, bass.ts(m, 128)],
                                             qT[:, k, :], start=(k == 0),
                                             stop=(k == KT - 1))
                        # y = (psum + bo) + x   (into kTt, reused as y)
                        nc.vector.scalar_tensor_tensor(
                            kTt[:, m, :], p_o[:], bo_t[:, l, m:m + 1],
                            xT[:, m, :], op0=ALU.add, op1=ALU.add)
                        ln_sums(ps, kTt, m, m == 0, m == MT - 1, st_ps)
                    ln_finalize(ps, kTt, g1_t, b1_t, l, attnT, st_ps, warm=6)

                # ================= FFN =================
                with tc.tile_pool(name="ps_ffn", bufs=1, space="PSUM") as ps:
                    ffo = [ps.tile([128, S], F32, tag="ffo", name=f"ffo{m}",
                                   bufs=6)
                           for m in range(MT)]
                    # software-pipelined: p_f chain of ko+1 issues before the
                    # ffo accumulation of ko so the PE isn't starved while the
                    # gelu of ko drains.
                    ffts = {}
                    pf01 = []

                    def ffn1_weights(ko):
                        wi_t = wf1pool.tile([128, KT, 128], WDT, tag="wff1",
                                            name=f"wi{ko}")
                        nc.sync.dma_start(out=wi_t[:], in_=WI.ap()[l, ko])
                        wf_t = wpool.tile([128, H], WDT, tag="w768",
                                          name=f"wf{ko}")
                        nc.sync.dma_start(out=wf_t[:],
                                          in_=WF.ap()[l, bass.ts(ko, 128), :])
                        return wi_t, wf_t

                    def ffn1_finish(ko, wi_t, wf_t, p_f):
                        ff_t = sb.tile([128, S], ADT, tag="fft", name="fft",
                                       bufs=4)
                        nc.scalar.activation(ff_t[:], p_f[:], AF.Gelu,
                                             bias=bi_t[:, l, ko:ko + 1])
                        ffts[ko] = (ff_t, wf_t)

                    for ko in range(IT + 1):
                        if ko < IT:
                            wi_t, wf_t = ffn1_weights(ko)
                            p_f = ps.tile([128, S], F32, tag="ff1", name="pf",
                                          bufs=2)
                            for k in range(KT):
                                nc.tensor.matmul(p_f[:], wi_t[:, k, :],
                                                 attnT[:, k, :],
                                                 start=(k == 0),
                                                 stop=(k == KT - 1))
                            ffn1_finish(ko, wi_t, wf_t, p_f)
                        if ko >= 1:
                            ff_p, wf_p = ffts.pop(ko - 1)
                            for m in range(MT):
                                nc.tensor.matmul(ffo[m][:],
                                                 wf_p[:, bass.ts(m, 128)],
                                                 ff_p[:], start=(ko - 1 == 0),
                                                 stop=(ko - 1 == IT - 1))
                    # dummy 1-elem ln: hoists the natural_log_exp-set
                    # ACT_TABLE_LOAD off LN2's serial chain (Act is idle
                    # after the last gelu while FFN2 drains).
                    scr2 = sb.tile([1, 1], F32, tag="scr", name="scr2", bufs=2)
                    nc.scalar.activation(scr2[:], eps_t[:], AF.Ln)
                    for m in range(MT):
                        # y2 = (ffo + bf) + attnT   (into kTt)
                        nc.vector.scalar_tensor_tensor(
                            kTt[:, m, :], ffo[m][:], bf_t[:, l, m:m + 1],
                            attnT[:, m, :], op0=ALU.add, op1=ALU.add)
                with tc.tile_pool(name="ps_ln2", bufs=1, space="PSUM") as ps:
                    st_ps = ps.tile([1, 2, S], F32, tag="sum", name="sum")
                    ln_sums_split(ps, kTt, st_ps)
                    # On the last pass the LN2 normalize writes the fp32
                    # output tile directly (skips a 3us full-width copy).
                    ln_finalize(ps, kTt, g2_t, b2_t, l,
                                xout if last else xT, st_ps, warm=6)

            xout = pers.tile([128, KT, S], F32, tag="xout")
            for r in range(repeat):
                for l in range(n_layers):
                    layer_body(l, last=(r == repeat - 1 and l == n_layers - 1))

            nc.sync.dma_start(
                out=OUT.ap().rearrange("(k p) s -> p k s", p=128),
                in_=xout[:])

    nc.compile()
    return nc


_CACHE = {}


def get_program(repeat=1, n_layers=L):
    key = (repeat, n_layers)
    if key not in _CACHE:
        _CACHE[key] = build_program(repeat, n_layers)
    return _CACHE[key]


def make_input_maps(inputs):
    """Per-core input maps from the full-batch input dict."""
    import ml_dtypes
    wnp = ml_dtypes.bfloat16 if WDT == BF16 else np.float32
    anp = ml_dtypes.bfloat16 if ADT == BF16 else np.float32
    hs = np.ascontiguousarray(np.asarray(inputs["hidden_states"], np.float32))
    mask = np.asarray(inputs["attention_mask"], np.float32)
    wi = np.ascontiguousarray(
        np.asarray(inputs["Wi"], np.float32).reshape(L, KT, 128, IT, 128)
        .transpose(0, 3, 2, 1, 4)).astype(wnp)
    shared = {
        "WQ": np.ascontiguousarray(np.asarray(inputs["Wq"], np.float32)).astype(wnp),
        "WK": np.ascontiguousarray(np.asarray(inputs["Wk"], np.float32)).astype(wnp),
        "WV": np.ascontiguousarray(np.asarray(inputs["Wv"], np.float32)).astype(wnp),
        "WO": np.ascontiguousarray(np.asarray(inputs["Wo"], np.float32)).astype(wnp),
        "WI": wi,
        "WF": np.ascontiguousarray(np.asarray(inputs["Wf"], np.float32)).astype(wnp),
        "BVB": np.asarray(inputs["bv"], np.float32).astype(anp),
        "PP8": np.ascontiguousarray(np.stack(
            [np.asarray(inputs[k], np.float32).reshape(L, KT, 128)
             .transpose(2, 0, 1)
             for k in ("bq", "bk", "bo", "bf", "ln1_g", "ln1_b",
                       "ln2_g", "ln2_b")], axis=1)),
        "PBI": np.ascontiguousarray(
            np.asarray(inputs["bi"], np.float32).reshape(L, IT, 128)
            .transpose(2, 0, 1)),
    }
    in_maps = []
    for c in range(B):
        ext = ((1.0 - mask[c]) * -10000.0).astype(np.float32).reshape(ST, 128)
        in_maps.append({
            "XT": np.ascontiguousarray(hs[c].T).astype(anp),
            "EXTM": ext,
            **shared,
        })
    return in_maps


def kernel(**inputs):
    nc = get_program(repeat=1)
    in_maps = make_input_maps(inputs)
    res = run_bass_kernel_spmd(nc, in_maps, list(range(B)))
    out = np.stack([res.results[c]["OUT"].T for c in range(B)], axis=0)
    return out.astype(np.float32)


# revision 69
# speedup vs baseline: 1492.7382x; 1.0023x over previous
"""BERT-base 12-layer encoder forward on 8 trn2 NeuronCores.

Strategy: pure data parallelism — batch B=8, one sequence per core, full
weights replicated (bf16 in HBM, halving weight DMA), zero collectives.
All matmul operands (weights AND activations) are bf16 — the PE runs
bf16 at the same 1 column/cycle as fp32r, so this costs nothing on the
tensor engine while halving DMA and SBUF traffic; accumulation, LN
statistics, and softmax denominators stay fp32 in PSUM. Activations are
feature-major ([hidden, seq], hidden on SBUF partitions) so every
projection is a PE matmul with the weight stationary.

v2 changes vs baseline:
- QKV projections for head-pair p+1 are interleaved with the
  scores/exp/PV of pair p, so the Act engine's softmax-exp stream
  (~27us/layer, the attention bottleneck) hides behind the PE's
  projection matmuls instead of serializing after them.
- Softmax 1/denom uses reciprocal_approx_fast (1 DVE pass, ~18-bit)
  instead of the iterative-divide reciprocal (8 cyc/elem on a single
  partition lane).
- LayerNorm rstd = exp(-0.5*ln(var+eps)) on the Act engine: ln and exp
  live in the same activation-table set as the softmax exp
  (natural_log_exp_and_others), so the per-layer sqrt-set switches
  (~2.7us each, 2/layer) and the slow DVE reciprocal disappear.
"""
import sys

sys.path.insert(0, "/opt/trn_rl_repo")

import numpy as np
import concourse.bass as bass
import concourse.mybir as mybir
import concourse.tile as tile
from concourse import bacc
from concourse.bass_utils import run_bass_kernel_spmd
from concourse.dve_ops import RECIPROCAL_APPROX_FAST, RECIP_APPROX_FAST_CONSTS

F32 = mybir.dt.float32
F32R = mybir.dt.float32r
BF16 = mybir.dt.bfloat16
AF = mybir.ActivationFunctionType
ALU = mybir.AluOpType

L, H, NH, I = 12, 768, 12, 3072
DH = 64
B, S = 8, 512
KT = H // 128          # 6 k-tiles over hidden
MT = H // 128          # 6 m-tiles over hidden
IT = I // 128          # 24 tiles over intermediate
ST = S // 128          # 4 tiles over sequence
NP = NH // 2           # 6 head pairs
EPS = 1e-12
SCALE = 0.125          # 1/sqrt(64)
RSQH = 1.0 / float(np.sqrt(H))
WDT = BF16             # weight dtype in HBM
ADT = BF16             # activation dtype (matmul operands)


def build_program(repeat=1, n_layers=L):
    # Build-time activation-table steering: the default greedy set picker
    # chooses exp_and_others for Exp and natural_log for Ln, costing 6
    # ACT_TABLE_LOADs (~2.7us each on HW) per layer. Restricting the
    # visible sets to natural_log_exp_and_others (exp+ln+square+identity)
    # and gelu_and_others makes every layer need exactly 2 loads
    # (exp-set <-> gelu-set around the FFN). Indices are preserved, so the
    # emitted act_func_set_id still refers to the real act_info.json
    # entries. Restored right after the build.
    import concourse.hw_specs as hw_specs
    _orig_gat = hw_specs.get_activation_tables
    _keep = {"natural_log_exp_and_others", "gelu_and_others"}

    def _patched_gat(module_arch):
        tabs = _orig_gat(module_arch)
        return {name: (fns if name in _keep else set())
                for name, fns in tabs.items()}

    hw_specs.get_activation_tables = _patched_gat
    import concourse.bacc as bacc_mod
    _bacc_had = getattr(bacc_mod, "get_activation_tables", None)
    if _bacc_had is not None:
        bacc_mod.get_activation_tables = _patched_gat
    try:
        return _build_program_inner(repeat, n_layers)
    finally:
        hw_specs.get_activation_tables = _orig_gat
        if _bacc_had is not None:
            bacc_mod.get_activation_tables = _bacc_had


def _build_program_inner(repeat=1, n_layers=L):
    nc = bacc.Bacc("TRN2", target_bir_lowering=False)

    XT = nc.dram_tensor("XT", [H, S], ADT, kind="ExternalInput")
    EXTM = nc.dram_tensor("EXTM", [ST, 128], F32, kind="ExternalInput")
    WQ = nc.dram_tensor("WQ", [L, H, H], WDT, kind="ExternalInput")
    WK = nc.dram_tensor("WK", [L, H, H], WDT, kind="ExternalInput")
    WV = nc.dram_tensor("WV", [L, H, H], WDT, kind="ExternalInput")
    WO = nc.dram_tensor("WO", [L, H, H], WDT, kind="ExternalInput")
    WI = nc.dram_tensor("WI", [L, IT, 128, KT, 128], WDT, kind="ExternalInput")
    WF = nc.dram_tensor("WF", [L, I, H], WDT, kind="ExternalInput")
    BVB = nc.dram_tensor("BVB", [L, H], ADT, kind="ExternalInput")
    # 8 per-layer param vectors pre-transposed host-side to the on-chip
    # feature-major layout [128, L, KT] and packed into one contiguous
    # tensor (one big DMA instead of 9 scatter-pattern DMAs with ~72
    # descriptors per partition each).
    PP8 = nc.dram_tensor("PP8", [128, 8, L, KT], F32, kind="ExternalInput")
    PBI = nc.dram_tensor("PBI", [128, L, IT], F32, kind="ExternalInput")
    OUT = nc.dram_tensor("OUT", [H, S], F32, kind="ExternalOutput")
    import os
    KDBG = os.environ.get("KDBG") == "1"
    if KDBG:
        DQ = nc.dram_tensor("DQ", [H, S], F32, kind="ExternalOutput")
        DK = nc.dram_tensor("DK", [H, S], F32, kind="ExternalOutput")

    with tile.TileContext(nc) as tc:
        with (
            nc.allow_low_precision(reason="bf16 matmul pipeline"),
            tc.tile_pool(name="pers", bufs=1) as pers,
            tc.tile_pool(name="w768", bufs=26) as wpool,
            tc.tile_pool(name="wff1", bufs=4) as wf1pool,
            tc.tile_pool(name="sb", bufs=2) as sb,
        ):
            # ---- persistent activations ----
            xT = pers.tile([128, KT, S], ADT, tag="xT")
            nc.sync.dma_start(out=xT[:], in_=XT.ap().rearrange(
                "(k p) s -> p k s", p=128))
            qT = pers.tile([128, KT, S], ADT, tag="qT")    # reused as ctxT
            kTt = pers.tile([128, KT, S], ADT, tag="kTt")  # reused as LN input y
            attnT = pers.tile([128, KT, S], ADT, tag="attnT")
            v_aug = pers.tile([128, ST, NH, DH + 1], ADT, tag="vaug")
            nc.vector.memset(v_aug[:, :, :, DH], 1.0)

            ext = pers.tile([128, ST], F32, tag="ext")
            nc.sync.dma_start(out=ext[:], in_=EXTM.ap().rearrange("k p -> p k"))

            # ---- constants ----
            ones128c = pers.tile([1, 128], ADT, tag="ones128c")
            nc.vector.memset(ones128c[:], 1.0)
            invh128c = pers.tile([1, 128], ADT, tag="invh128c")
            nc.vector.memset(invh128c[:], 1.0 / H)
            ones128p = pers.tile([128, 1], ADT, tag="ones128p")
            nc.vector.memset(ones128p[:], 1.0)
            ones128pc = pers.tile([1, 128], ADT, tag="ones128pc")
            nc.vector.memset(ones128pc[:], 1.0)
            ones64 = pers.tile([1, DH], ADT, tag="ones64")
            nc.vector.memset(ones64[:], 1.0)
            eps_t = pers.tile([1, 1], F32, tag="eps")
            nc.vector.memset(eps_t[:], EPS)

            # ---- per-layer params, loaded once (feature-major [128, L, KT],
            # pre-transposed host-side; single contiguous DMA) ----
            pp8 = pers.tile([128, 8, L, KT], F32, tag="pp8")
            nc.sync.dma_start(out=pp8[:], in_=PP8.ap())
            bi_t = pers.tile([128, L, IT], F32, tag="pbi")
            nc.sync.dma_start(out=bi_t[:], in_=PBI.ap())
            bq_t = pp8[:, 0]; bk_t = pp8[:, 1]
            bo_t = pp8[:, 2]; bf_t = pp8[:, 3]
            g1_t = pp8[:, 4]; b1_t = pp8[:, 5]
            g2_t = pp8[:, 6]; b2_t = pp8[:, 7]

            def ln_sums(ps, y, k, first, last, st_ps):
                """Accumulate sum / sumsq of y k-tile into st_ps rows.
                Squares ride the idle GPSIMD engine so the Act engine's
                queue stays clear for the stats chain that follows."""
                nc.tensor.matmul(st_ps[:, 0, :], ones128p[:], y[:, k, :],
                                 start=first, stop=last)
                sq = sb.tile([128, S], ADT, tag="sq", name="sq", bufs=4)
                nc.gpsimd.tensor_mul(sq[:], y[:, k, :], y[:, k, :])
                nc.tensor.matmul(st_ps[:, 1, :], ones128p[:], sq[:],
                                 start=first, stop=last)

            def ln_sums_split(ps, y, st_ps):
                """Sums in separate pipelined loops (for a phase tail where
                the PE has no other work to hide per-tile latencies).
                Squares alternate gpsimd/Act: the Act engine is idle after
                the last gelu, and gpsimd alone was the ~6.4us long pole."""
                for k in range(KT):
                    nc.tensor.matmul(st_ps[:, 0, :], ones128p[:], y[:, k, :],
                                     start=(k == 0), stop=(k == KT - 1))
                for k in range(KT):
                    sq = sb.tile([128, S], ADT, tag="sq", name="sq", bufs=4)
                    if k % 2 == 0:
                        nc.gpsimd.tensor_mul(sq[:], y[:, k, :], y[:, k, :])
                    else:
                        nc.scalar.activation(sq[:], y[:, k, :], AF.Square)
                    nc.tensor.matmul(st_ps[:, 1, :], ones128p[:], sq[:],
                                     start=(k == 0), stop=(k == KT - 1))

            def ln_finalize(ps, y, gam, bet, l, out, st_ps, warm=0,
                            preload_gelu=False):
                """LN stats + normalize.

                rstd = exp(-0.5*ln(varh/H + eps)) keeps the whole chain in
                the natural_log_exp activation-table set (no sqrt-set
                switch, no slow DVE reciprocal).

                `warm` > 0 issues dummy ones-matmuls into a scratch PSUM
                bank while the stats chain runs — the PE p-state drops to
                1.2GHz after a ~3.4us idle gap, so keeping it streaming
                through this serial chain makes the next phase's matmuls
                start at full clock."""
                if warm:
                    w_ps = ps.tile([1, S], F32, tag="warm", name="warm")
                    for w in range(warm):
                        nc.tensor.matmul(w_ps[:], ones128p[:], y[:, w % KT, :],
                                         start=True, stop=True)
                sums = sb.tile([1, S], ADT, tag="sums", name="sums", bufs=2)
                nc.vector.tensor_copy(sums[:], st_ps[:, 0, :])
                # s2h = (sum/sqrt(H))^2 straight from PSUM on Act
                s2h = sb.tile([1, S], F32, tag="s2h", name="s2h", bufs=2)
                nc.scalar.activation(s2h[:], st_ps[:, 0, :], AF.Square,
                                     scale=RSQH)
                varh = sb.tile([1, S], F32, tag="varh", name="varh", bufs=2)
                nc.vector.tensor_sub(varh[:], st_ps[:, 1, :], s2h[:])
                lnv = sb.tile([1, S], F32, tag="lnv", name="lnv", bufs=2)
                nc.scalar.activation(lnv[:], varh[:], AF.Ln,
                                     bias=eps_t[:], scale=1.0 / H)
                rstd = sb.tile([1, S], ADT, tag="rstd", name="rstd", bufs=2)
                nc.scalar.activation(rstd[:], lnv[:], AF.Exp, scale=-0.5)
                mub_ps = ps.tile([128, S], F32, tag="mub", name="mub", bufs=1)
                nc.tensor.matmul(mub_ps[:], invh128c[:], sums[:],
                                 start=True, stop=True)
                rsb_ps = ps.tile([128, S], F32, tag="rsb", name="rsb", bufs=1)
                nc.tensor.matmul(rsb_ps[:], ones128c[:], rstd[:],
                                 start=True, stop=True)
                if warm:
                    for w in range(warm):
                        nc.tensor.matmul(w_ps[:], ones128p[:], y[:, w % KT, :],
                                         start=True, stop=True)
                # bf16 SBUF copies of the broadcasts: the GPSIMD half needs
                # SBUF (no PSUM port), and bf16 operands put the DVE
                # tensor_tensor ops in 2x mode (424ns vs 690ns a tile).
                # Mean/rstd in bf16 cost ~1e-3 relative — well inside
                # tolerance. DVE tile 0 reads PSUM directly so the first
                # output tile doesn't wait for the copies.
                mub_sb = sb.tile([128, S], ADT, tag="mubsb", name="mubsb", bufs=2)
                nc.scalar.copy(mub_sb[:], mub_ps[:])
                rsb_sb = sb.tile([128, S], ADT, tag="rsbsb", name="rsbsb", bufs=2)
                nc.scalar.copy(rsb_sb[:], rsb_ps[:])
                for k in range(KT):
                    dve = k % 2 == 0
                    eng = nc.vector if dve else nc.gpsimd
                    first = dve and k == 0
                    mu_in = mub_ps[:] if first else mub_sb[:]
                    rs_in = rsb_ps[:] if first else rsb_sb[:]
                    t1 = sb.tile([128, S], F32 if first else ADT, tag="lnt1",
                                 name="lnt1", bufs=4)
                    eng.tensor_sub(t1[:], y[:, k, :], mu_in)
                    t2 = sb.tile([128, S], ADT, tag="lnt2", name="lnt2", bufs=4)
                    eng.tensor_mul(t2[:], t1[:], rs_in)
                    nc.scalar.activation(out[:, k, :], t2[:], AF.Identity,
                                         bias=bet[:, l, k:k + 1],
                                         scale=gam[:, l, k:k + 1])
                    if preload_gelu and k == 0:
                        # dummy 1-elem gelu right after the first normalize
                        # identity: the gelu-set ACT_TABLE_LOAD lands in the
                        # Act engine's slack between identity tiles (paced
                        # by the DVE/gpsimd normalize) instead of on the
                        # FFN critical path. Later identities exist in the
                        # gelu set too, so no reload.
                        scr = sb.tile([1, 1], F32, tag="scr", name="scr",
                                      bufs=2)
                        nc.scalar.activation(scr[:], eps_t[:], AF.Gelu)

            def load_w768(dram, l, k, name):
                w = wpool.tile([128, H], WDT, tag="w768", name=name)
                nc.sync.dma_start(out=w[:], in_=dram.ap()[l, bass.ts(k, 128), :])
                return w

            def layer_body(l, last=False):
                # ============ QKV + attention, pair-pipelined ============
                # Q is projected first in two k-outer groups of 3 m-tiles
                # (3 PSUM banks): the k-outer order lets each matmul start
                # as soon as the previous layer's LN2 normalize emits that
                # xT k-tile, filling the LN2 chain's PE-idle window.
                # Head-pair m then needs exactly the m-th K-projection
                # tile, so K is computed per-pair inside the softmax
                # pipeline: while the Act engine streams exp tiles for
                # pair p+1, the PE runs K_{p+1} and PV_p.
                wq = [load_w768(WQ, l, k, f"wq{k}") for k in range(KT)]
                wk = [load_w768(WK, l, k, f"wk{k}") for k in range(KT)]
                wv = [load_w768(WV, l, k, f"wv{k}") for k in range(KT)]

                def proj_qk(ps, m):
                    p_q = ps.tile([128, S], F32, tag="qk", name="pq", bufs=2)
                    for k in range(KT):
                        nc.tensor.matmul(p_q[:], wq[k][:, bass.ts(m, 128)],
                                         xT[:, k, :], start=(k == 0),
                                         stop=(k == KT - 1))
                    nc.scalar.activation(qT[:, m, :], p_q[:], AF.Identity,
                                         bias=bq_t[:, l, m:m + 1])
                    p_k = ps.tile([128, S], F32, tag="qk", name="pk", bufs=2)
                    for k in range(KT):
                        nc.tensor.matmul(p_k[:], wk[k][:, bass.ts(m, 128)],
                                         xT[:, k, :], start=(k == 0),
                                         stop=(k == KT - 1))
                    nc.scalar.activation(kTt[:, m, :], p_k[:], AF.Identity,
                                         bias=bk_t[:, l, m:m + 1])

                def scores_exp(ps, pr):
                    exps = {}
                    for hh in range(2):
                        p0 = hh * DH
                        tp = None if hh == 0 else (64, 0)
                        for m in range(ST):
                            s_ps = ps.tile([128, S], F32, tag="scores",
                                           name="sps", bufs=2)
                            nc.tensor.matmul(
                                s_ps[:],
                                kTt[p0:p0 + DH, pr, bass.ts(m, 128)],
                                qT[p0:p0 + DH, pr, :],
                                start=True, stop=True, tile_position=tp)
                            e_t = sb.tile([128, S], ADT, tag="exp",
                                          name="expt", bufs=16)
                            nc.scalar.activation(e_t[:], s_ps[:], AF.Exp,
                                                 bias=ext[:, m:m + 1],
                                                 scale=SCALE)
                            exps[(hh, m)] = e_t
                    return exps

                def pv_split(ps, pr, exps):
                    # Last-pair variant: per-head denominator copy/recip so
                    # head 0's chain runs while head 1's PV matmuls are
                    # still accumulating — shortens the attention tail the
                    # Wo phase waits on.
                    c2 = ps.tile([128, 2, S], F32, tag="ctx", name="cps",
                                 bufs=2)
                    c = RECIP_APPROX_FAST_CONSTS
                    rcp2 = sb.tile([1, 2, S], ADT, tag="rcp2", name="rcp2",
                                   bufs=3)
                    for hh in range(2):
                        h = 2 * pr + hh
                        for m in range(ST):
                            nc.tensor.matmul(c2[0:DH + 1, hh, :],
                                             v_aug[:, m, h, :],
                                             exps[(hh, m)][:],
                                             start=(m == 0), stop=(m == ST - 1))
                        den = sb.tile([1, S], F32, tag="den1", name="den1",
                                      bufs=2)
                        nc.vector.tensor_copy(den[:], c2[DH:DH + 1, hh, :])
                        nc.vector._custom_dve(
                            RECIPROCAL_APPROX_FAST, out=rcp2[:, hh, :],
                            in0=den[:], s0=c["s0"], s1=c["s1"],
                            imm2=c["imm2"])
                    return (pr, c2, rcp2)

                def pv(ps, pr, exps):
                    # Both heads' PV into one 2-bank PSUM tile so the two
                    # softmax denominator rows (partition 64) sit in one
                    # contiguous free range: one DVE copy to SBUF, then the
                    # custom-DVE fast reciprocal (bf16 out). The custom op
                    # reads garbage from PSUM on real HW (SBUF-only), hence
                    # the copy.
                    c2 = ps.tile([128, 2, S], F32, tag="ctx", name="cps",
                                 bufs=2)
                    for hh in range(2):
                        h = 2 * pr + hh
                        for m in range(ST):
                            nc.tensor.matmul(c2[0:DH + 1, hh, :],
                                             v_aug[:, m, h, :],
                                             exps[(hh, m)][:],
                                             start=(m == 0), stop=(m == ST - 1))
                    den = sb.tile([1, 2, S], F32, tag="den", name="den",
                                  bufs=3)
                    nc.vector.tensor_copy(den[:], c2[DH:DH + 1, :, :])
                    rcp2 = sb.tile([1, 2, S], ADT, tag="rcp2", name="rcp2",
                                   bufs=3)
                    c = RECIP_APPROX_FAST_CONSTS
                    nc.vector._custom_dve(
                        RECIPROCAL_APPROX_FAST,
                        out=rcp2[:].rearrange("p a s -> p (a s)"),
                        in0=den[:].rearrange("p a s -> p (a s)"),
                        s0=c["s0"], s1=c["s1"], imm2=c["imm2"])
                    return (pr, c2, rcp2)

                def att_finalize(ps, pend):
                    # bc_ps shares the scores pool rotation (no extra bank)
                    pr_, c2_, rcp2_ = pend
                    bc_ps = ps.tile([128, S], F32, tag="scores", name="bcps",
                                    bufs=2)
                    for hh in range(2):
                        nc.tensor.matmul(bc_ps[hh * DH:(hh + 1) * DH, :],
                                         ones64[:], rcp2_[:, hh, :],
                                         start=True, stop=True)
                    bc_sb = sb.tile([128, S], ADT, tag="bcsb", name="bcsb",
                                    bufs=2)
                    nc.vector.tensor_copy(bc_sb[:], bc_ps[:])
                    for hh in range(2):
                        p0 = hh * DH
                        nc.vector.tensor_mul(
                            qT[p0:p0 + DH, pr_, :],
                            c2_[0:DH, hh, :],
                            bc_sb[p0:p0 + DH, :])

                with tc.tile_pool(name="ps_qk", bufs=1, space="PSUM") as psqk:
                    # pair 0 projections + scores, then V while its exps run
                    proj_qk(psqk, 0)
                    with tc.tile_pool(name="ps_sc", bufs=1,
                                      space="PSUM") as pssc:
                        exps0 = scores_exp(pssc, 0)
                        # V projection (seq-major, bias via K=1 ones matmul)
                        bv_row = sb.tile([1, H], ADT, tag="bvrow",
                                         name="bvrow", bufs=2)
                        nc.sync.dma_start(out=bv_row[:], in_=BVB.ap()[l:l + 1, :])
                        with tc.tile_pool(name="ps_v", bufs=1,
                                          space="PSUM") as psv:
                            for s in range(ST):
                                p_a = psv.tile([128, S], F32, tag="va",
                                               name="pva", bufs=2)
                                p_b = psv.tile([128, 256], F32, tag="vb",
                                               name="pvb", bufs=2)
                                for k in range(KT):
                                    nc.tensor.matmul(
                                        p_a[:], xT[:, k, bass.ts(s, 128)],
                                        wv[k][:, 0:512],
                                        start=(k == 0), stop=False)
                                    nc.tensor.matmul(
                                        p_b[:], xT[:, k, bass.ts(s, 128)],
                                        wv[k][:, 512:768],
                                        start=(k == 0), stop=False)
                                nc.tensor.matmul(p_a[:], ones128pc[:],
                                                 bv_row[:, 0:512],
                                                 start=False, stop=True)
                                nc.tensor.matmul(p_b[:], ones128pc[:],
                                                 bv_row[:, 512:768],
                                                 start=False, stop=True)
                                nc.vector.tensor_copy(
                                    v_aug[:, s, 0:8, 0:DH],
                                    p_a[:].rearrange("p (h c) -> p h c", c=DH))
                                nc.vector.tensor_copy(
                                    v_aug[:, s, 8:12, 0:DH],
                                    p_b[:].rearrange("p (h c) -> p h c", c=DH))
                        # steady pair pipeline
                        with tc.tile_pool(name="ps_ctx", bufs=1,
                                          space="PSUM") as psctx:
                            pending = None
                            exps = exps0
                            for pr in range(NP):
                                if pr + 1 < NP:
                                    proj_qk(psqk, pr + 1)
                                # finalize pair pr-1 now: its reciprocal was
                                # issued a full pair-iteration ago, so the
                                # broadcast matmuls at the PE queue head
                                # never wait on the DVE chain — and its
                                # scores-pool slot was drained by exps(pr)
                                # long ago.
                                if pending is not None:
                                    att_finalize(pssc, pending)
                                nxt = (scores_exp(pssc, pr + 1)
                                       if pr + 1 < NP else None)
                                pending = pv_split(psctx, pr, exps)
                                exps = nxt
                            # last pair: per-head broadcast/normalize so
                            # head 0's qT tile lands while head 1's chain
                            # still runs (Wo waits on this tail).
                            pr_, c2_, rcp2_ = pending
                            bc_ps = pssc.tile([128, S], F32, tag="scores",
                                              name="bcps", bufs=2)
                            bc_sb = sb.tile([128, S], ADT, tag="bcsb",
                                            name="bcsb", bufs=2)
                            for hh in range(2):
                                p0 = hh * DH
                                nc.tensor.matmul(bc_ps[p0:p0 + DH, :],
                                                 ones64[:], rcp2_[:, hh, :],
                                                 start=True, stop=True)
                                nc.vector.tensor_copy(bc_sb[p0:p0 + DH, :],
                                                      bc_ps[p0:p0 + DH, :])
                                nc.vector.tensor_mul(
                                    qT[p0:p0 + DH, pr_, :],
                                    c2_[0:DH, hh, :],
                                    bc_sb[p0:p0 + DH, :])

                if KDBG and l == 0:
                    dq = pers.tile([128, KT, S], F32, tag="dbgq")
                    nc.vector.tensor_copy(dq[:], qT[:])
                    nc.sync.dma_start(
                        out=DQ.ap().rearrange("(k p) s -> p k s", p=128),
                        in_=dq[:])
                    dk = pers.tile([128, KT, S], F32, tag="dbgk")
                    nc.vector.tensor_copy(dk[:], kTt[:])
                    nc.sync.dma_start(
                        out=DK.ap().rearrange("(k p) s -> p k s", p=128),
                        in_=dk[:])

                # ================= Wo + residual + LN1 =================
                with tc.tile_pool(name="ps_wo", bufs=1, space="PSUM") as ps:
                    wo = [load_w768(WO, l, k, f"wo{k}") for k in range(KT)]
                    st_ps = ps.tile([1, 2, S], F32, tag="sum", name="sum")
                    for m in range(MT):
                        p_o = ps.tile([128, S], F32, tag="proj", name="po",
                                      bufs=3)
                        for k in range(KT):
                            nc.tensor.matmul(p_o[:], wo[k][:, bass.ts(m, 128)],
                                             qT[:, k, :], start=(k == 0),
                                             stop=(k == KT - 1))
                        # y = (psum + bo) + x   (into kTt, reused as y)
                        nc.vector.scalar_tensor_tensor(
                            kTt[:, m, :], p_o[:], bo_t[:, l, m:m + 1],
                            xT[:, m, :], op0=ALU.add, op1=ALU.add)
                        ln_sums(ps, kTt, m, m == 0, m == MT - 1, st_ps)
                    ln_finalize(ps, kTt, g1_t, b1_t, l, attnT, st_ps, warm=6)

                # ================= FFN =================
                with tc.tile_pool(name="ps_ffn", bufs=1, space="PSUM") as ps:
                    ffo = [ps.tile([128, S], F32, tag="ffo", name=f"ffo{m}",
                                   bufs=6)
                           for m in range(MT)]
                    # software-pipelined: p_f chain of ko+1 issues before the
                    # ffo accumulation of ko so the PE isn't starved while the
                    # gelu of ko drains.
                    ffts = {}
                    pf01 = []

                    def ffn1_weights(ko):
                        wi_t = wf1pool.tile([128, KT, 128], WDT, tag="wff1",
                                            name=f"wi{ko}")
                        nc.sync.dma_start(out=wi_t[:], in_=WI.ap()[l, ko])
                        wf_t = wpool.tile([128, H], WDT, tag="w768",
                                          name=f"wf{ko}")
                        nc.sync.dma_start(out=wf_t[:],
                                          in_=WF.ap()[l, bass.ts(ko, 128), :])
                        return wi_t, wf_t

                    def ffn1_finish(ko, wi_t, wf_t, p_f):
                        ff_t = sb.tile([128, S], ADT, tag="fft", name="fft",
                                       bufs=4)
                        nc.scalar.activation(ff_t[:], p_f[:], AF.Gelu,
                                             bias=bi_t[:, l, ko:ko + 1])
                        ffts[ko] = (ff_t, wf_t)

                    for ko in range(IT + 1):
                        if ko < IT:
                            wi_t, wf_t = ffn1_weights(ko)
                            p_f = ps.tile([128, S], F32, tag="ff1", name="pf",
                                          bufs=2)
                            for k in range(KT):
                                nc.tensor.matmul(p_f[:], wi_t[:, k, :],
                                                 attnT[:, k, :],
                                                 start=(k == 0),
                                                 stop=(k == KT - 1))
                            ffn1_finish(ko, wi_t, wf_t, p_f)
                        if ko >= 1:
                            ff_p, wf_p = ffts.pop(ko - 1)
                            for m in range(MT):
                                nc.tensor.matmul(ffo[m][:],
                                                 wf_p[:, bass.ts(m, 128)],
                                                 ff_p[:], start=(ko - 1 == 0),
                                                 stop=(ko - 1 == IT - 1))
                    # dummy 1-elem ln: hoists the natural_log_exp-set
                    # ACT_TABLE_LOAD off LN2's serial chain (Act is idle
                    # after the last gelu while FFN2 drains).
                    scr2 = sb.tile([1, 1], F32, tag="scr", name="scr2", bufs=2)
                    nc.scalar.activation(scr2[:], eps_t[:], AF.Ln)
                    for m in range(MT):
                        # y2 = (ffo + bf) + attnT   (into kTt)
                        nc.vector.scalar_tensor_tensor(
                            kTt[:, m, :], ffo[m][:], bf_t[:, l, m:m + 1],
                            attnT[:, m, :], op0=ALU.add, op1=ALU.add)
                with tc.tile_pool(name="ps_ln2", bufs=1, space="PSUM") as ps:
                    st_ps = ps.tile([1, 2, S], F32, tag="sum", name="sum")
                    ln_sums_split(ps, kTt, st_ps)
                    # On the last pass the LN2 normalize writes the fp32
                    # output tile directly (skips a 3us full-width copy).
                    ln_finalize(ps, kTt, g2_t, b2_t, l,
                                xout if last else xT, st_ps, warm=6)

            xout = pers.tile([128, KT, S], F32, tag="xout")
            for r in range(repeat):
                for l in range(n_layers):
                    layer_body(l, last=(r == repeat - 1 and l == n_layers - 1))

            # per-k-tile output DMA: each slice ships as soon as its
            # final LN2 normalize identity lands, instead of waiting for
            # the whole tile.
            outr = OUT.ap().rearrange("(k p) s -> p k s", p=128)
            for k in range(KT):
                nc.sync.dma_start(out=outr[:, k, :], in_=xout[:, k, :])

    nc.compile()
    return nc


_CACHE = {}


def get_program(repeat=1, n_layers=L):
    key = (repeat, n_layers)
    if key not in _CACHE:
        _CACHE[key] = build_program(repeat, n_layers)
    return _CACHE[key]


def make_input_maps(inputs):
    """Per-core input maps from the full-batch input dict."""
    import ml_dtypes
    wnp = ml_dtypes.bfloat16 if WDT == BF16 else np.float32
    anp = ml_dtypes.bfloat16 if ADT == BF16 else np.float32
    hs = np.ascontiguousarray(np.asarray(inputs["hidden_states"], np.float32))
    mask = np.asarray(inputs["attention_mask"], np.float32)
    wi = np.ascontiguousarray(
        np.asarray(inputs["Wi"], np.float32).reshape(L, KT, 128, IT, 128)
        .transpose(0, 3, 2, 1, 4)).astype(wnp)
    shared = {
        "WQ": np.ascontiguousarray(np.asarray(inputs["Wq"], np.float32)).astype(wnp),
        "WK": np.ascontiguousarray(np.asarray(inputs["Wk"], np.float32)).astype(wnp),
        "WV": np.ascontiguousarray(np.asarray(inputs["Wv"], np.float32)).astype(wnp),
        "WO": np.ascontiguousarray(np.asarray(inputs["Wo"], np.float32)).astype(wnp),
        "WI": wi,
        "WF": np.ascontiguousarray(np.asarray(inputs["Wf"], np.float32)).astype(wnp),
        "BVB": np.asarray(inputs["bv"], np.float32).astype(anp),
        "PP8": np.ascontiguousarray(np.stack(
            [np.asarray(inputs[k], np.float32).reshape(L, KT, 128)
             .transpose(2, 0, 1)
             for k in ("bq", "bk", "bo", "bf", "ln1_g", "ln1_b",
                       "ln2_g", "ln2_b")], axis=1)),
        "PBI": np.ascontiguousarray(
            np.asarray(inputs["bi"], np.float32).reshape(L, IT, 128)
            .transpose(2, 0, 1)),
    }
    in_maps = []
    for c in range(B):
        ext = ((1.0 - mask[c]) * -10000.0).astype(np.float32).reshape(ST, 128)
        in_maps.append({
            "XT": np.ascontiguousarray(hs[c].T).astype(anp),
            "EXTM": ext,
            **shared,
        })
    return in_maps


def kernel(**inputs):
    nc = get_program(repeat=1)
    in_maps = make_input_maps(inputs)
    res = run_bass_kernel_spmd(nc, in_maps, list(range(B)))
    out = np.stack([res.results[c]["OUT"].T for c in range(B)], axis=0)
    return out.astype(np.float32)


# revision 70
# speedup vs baseline: 1516.7960x; 1.0161x over previous
"""BERT-base 12-layer encoder forward on 8 trn2 NeuronCores.

Strategy: pure data parallelism — batch B=8, one sequence per core, full
weights replicated (bf16 in HBM, halving weight DMA), zero collectives.
All matmul operands (weights AND activations) are bf16 — the PE runs
bf16 at the same 1 column/cycle as fp32r, so this costs nothing on the
tensor engine while halving DMA and SBUF traffic; accumulation, LN
statistics, and softmax denominators stay fp32 in PSUM. Activations are
feature-major ([hidden, seq], hidden on SBUF partitions) so every
projection is a PE matmul with the weight stationary.

v2 changes vs baseline:
- QKV projections for head-pair p+1 are interleaved with the
  scores/exp/PV of pair p, so the Act engine's softmax-exp stream
  (~27us/layer, the attention bottleneck) hides behind the PE's
  projection matmuls instead of serializing after them.
- Softmax 1/denom uses reciprocal_approx_fast (1 DVE pass, ~18-bit)
  instead of the iterative-divide reciprocal (8 cyc/elem on a single
  partition lane).
- LayerNorm rstd = exp(-0.5*ln(var+eps)) on the Act engine: ln and exp
  live in the same activation-table set as the softmax exp
  (natural_log_exp_and_others), so the per-layer sqrt-set switches
  (~2.7us each, 2/layer) and the slow DVE reciprocal disappear.
"""
import sys

sys.path.insert(0, "/opt/trn_rl_repo")

import numpy as np
import concourse.bass as bass
import concourse.mybir as mybir
import concourse.tile as tile
from concourse import bacc
from concourse.bass_utils import run_bass_kernel_spmd
from concourse.dve_ops import RECIPROCAL_APPROX_FAST, RECIP_APPROX_FAST_CONSTS

F32 = mybir.dt.float32
F32R = mybir.dt.float32r
BF16 = mybir.dt.bfloat16
AF = mybir.ActivationFunctionType
ALU = mybir.AluOpType

L, H, NH, I = 12, 768, 12, 3072
DH = 64
B, S = 8, 512
KT = H // 128          # 6 k-tiles over hidden
MT = H // 128          # 6 m-tiles over hidden
IT = I // 128          # 24 tiles over intermediate
ST = S // 128          # 4 tiles over sequence
NP = NH // 2           # 6 head pairs
EPS = 1e-12
SCALE = 0.125          # 1/sqrt(64)
RSQH = 1.0 / float(np.sqrt(H))
WDT = BF16             # weight dtype in HBM
ADT = BF16             # activation dtype (matmul operands)


def build_program(repeat=1, n_layers=L):
    # Build-time activation-table steering: the default greedy set picker
    # chooses exp_and_others for Exp and natural_log for Ln, costing 6
    # ACT_TABLE_LOADs (~2.7us each on HW) per layer. Restricting the
    # visible sets to natural_log_exp_and_others (exp+ln+square+identity)
    # and gelu_and_others makes every layer need exactly 2 loads
    # (exp-set <-> gelu-set around the FFN). Indices are preserved, so the
    # emitted act_func_set_id still refers to the real act_info.json
    # entries. Restored right after the build.
    import concourse.hw_specs as hw_specs
    _orig_gat = hw_specs.get_activation_tables
    _keep = {"natural_log_exp_and_others", "gelu_and_others"}

    def _patched_gat(module_arch):
        tabs = _orig_gat(module_arch)
        return {name: (fns if name in _keep else set())
                for name, fns in tabs.items()}

    hw_specs.get_activation_tables = _patched_gat
    import concourse.bacc as bacc_mod
    _bacc_had = getattr(bacc_mod, "get_activation_tables", None)
    if _bacc_had is not None:
        bacc_mod.get_activation_tables = _patched_gat
    try:
        return _build_program_inner(repeat, n_layers)
    finally:
        hw_specs.get_activation_tables = _orig_gat
        if _bacc_had is not None:
            bacc_mod.get_activation_tables = _bacc_had


def _build_program_inner(repeat=1, n_layers=L):
    nc = bacc.Bacc("TRN2", target_bir_lowering=False)

    XT = nc.dram_tensor("XT", [H, S], ADT, kind="ExternalInput")
    EXTM = nc.dram_tensor("EXTM", [ST, 128], F32, kind="ExternalInput")
    WQ = nc.dram_tensor("WQ", [L, H, H], WDT, kind="ExternalInput")
    WK = nc.dram_tensor("WK", [L, H, H], WDT, kind="ExternalInput")
    WV = nc.dram_tensor("WV", [L, H, H], WDT, kind="ExternalInput")
    WO = nc.dram_tensor("WO", [L, H, H], WDT, kind="ExternalInput")
    WI = nc.dram_tensor("WI", [L, IT, 128, KT, 128], WDT, kind="ExternalInput")
    WF = nc.dram_tensor("WF", [L, I, H], WDT, kind="ExternalInput")
    BVB = nc.dram_tensor("BVB", [L, H], ADT, kind="ExternalInput")
    # 8 per-layer param vectors pre-transposed host-side to the on-chip
    # feature-major layout [128, L, KT] and packed into one contiguous
    # tensor (one big DMA instead of 9 scatter-pattern DMAs with ~72
    # descriptors per partition each).
    PP8 = nc.dram_tensor("PP8", [128, 8, L, KT], F32, kind="ExternalInput")
    PBI = nc.dram_tensor("PBI", [128, L, IT], F32, kind="ExternalInput")
    OUT = nc.dram_tensor("OUT", [H, S], F32, kind="ExternalOutput")
    import os
    KDBG = os.environ.get("KDBG") == "1"
    if KDBG:
        DQ = nc.dram_tensor("DQ", [H, S], F32, kind="ExternalOutput")
        DK = nc.dram_tensor("DK", [H, S], F32, kind="ExternalOutput")

    with tile.TileContext(nc) as tc:
        with (
            nc.allow_low_precision(reason="bf16 matmul pipeline"),
            tc.tile_pool(name="pers", bufs=1) as pers,
            tc.tile_pool(name="w768", bufs=26) as wpool,
            tc.tile_pool(name="wff1", bufs=4) as wf1pool,
            tc.tile_pool(name="sb", bufs=2) as sb,
        ):
            # ---- persistent activations ----
            xT = pers.tile([128, KT, S], ADT, tag="xT")
            nc.sync.dma_start(out=xT[:], in_=XT.ap().rearrange(
                "(k p) s -> p k s", p=128))
            qT = pers.tile([128, KT, S], ADT, tag="qT")    # reused as ctxT
            kTt = pers.tile([128, KT, S], ADT, tag="kTt")  # reused as LN input y
            attnT = pers.tile([128, KT, S], ADT, tag="attnT")
            v_aug = pers.tile([128, ST, NH, DH + 1], ADT, tag="vaug")
            nc.vector.memset(v_aug[:, :, :, DH], 1.0)

            ext = pers.tile([128, ST], F32, tag="ext")
            nc.sync.dma_start(out=ext[:], in_=EXTM.ap().rearrange("k p -> p k"))

            # ---- constants ----
            ones128c = pers.tile([1, 128], ADT, tag="ones128c")
            nc.vector.memset(ones128c[:], 1.0)
            invh128c = pers.tile([1, 128], ADT, tag="invh128c")
            nc.vector.memset(invh128c[:], 1.0 / H)
            ones128p = pers.tile([128, 1], ADT, tag="ones128p")
            nc.vector.memset(ones128p[:], 1.0)
            ones128pc = pers.tile([1, 128], ADT, tag="ones128pc")
            nc.vector.memset(ones128pc[:], 1.0)
            ones64 = pers.tile([1, DH], ADT, tag="ones64")
            nc.vector.memset(ones64[:], 1.0)
            eps_t = pers.tile([1, 1], F32, tag="eps")
            nc.vector.memset(eps_t[:], EPS)

            # ---- per-layer params, loaded once (feature-major [128, L, KT],
            # pre-transposed host-side; single contiguous DMA) ----
            pp8 = pers.tile([128, 8, L, KT], F32, tag="pp8")
            nc.sync.dma_start(out=pp8[:], in_=PP8.ap())
            bi_t = pers.tile([128, L, IT], F32, tag="pbi")
            nc.sync.dma_start(out=bi_t[:], in_=PBI.ap())
            bq_t = pp8[:, 0]; bk_t = pp8[:, 1]
            bo_t = pp8[:, 2]; bf_t = pp8[:, 3]
            g1_t = pp8[:, 4]; b1_t = pp8[:, 5]
            g2_t = pp8[:, 6]; b2_t = pp8[:, 7]

            def ln_sums(ps, y, k, first, last, st_ps):
                """Accumulate sum / sumsq of y k-tile into st_ps rows.
                Squares ride the idle GPSIMD engine so the Act engine's
                queue stays clear for the stats chain that follows."""
                nc.tensor.matmul(st_ps[:, 0, :], ones128p[:], y[:, k, :],
                                 start=first, stop=last)
                sq = sb.tile([128, S], ADT, tag="sq", name="sq", bufs=4)
                nc.gpsimd.tensor_mul(sq[:], y[:, k, :], y[:, k, :])
                nc.tensor.matmul(st_ps[:, 1, :], ones128p[:], sq[:],
                                 start=first, stop=last)

            def ln_sums_split(ps, y, st_ps):
                """Sums in separate pipelined loops (for a phase tail where
                the PE has no other work to hide per-tile latencies).
                Squares alternate gpsimd/Act: the Act engine is idle after
                the last gelu, and gpsimd alone was the ~6.4us long pole."""
                for k in range(KT):
                    nc.tensor.matmul(st_ps[:, 0, :], ones128p[:], y[:, k, :],
                                     start=(k == 0), stop=(k == KT - 1))
                for k in range(KT):
                    sq = sb.tile([128, S], ADT, tag="sq", name="sq", bufs=4)
                    if k % 2 == 0:
                        nc.gpsimd.tensor_mul(sq[:], y[:, k, :], y[:, k, :])
                    else:
                        nc.scalar.activation(sq[:], y[:, k, :], AF.Square)
                    nc.tensor.matmul(st_ps[:, 1, :], ones128p[:], sq[:],
                                     start=(k == 0), stop=(k == KT - 1))

            def ln_finalize(ps, y, gam, bet, l, out, st_ps, warm=0,
                            preload_gelu=False):
                """LN stats + normalize.

                rstd = exp(-0.5*ln(varh/H + eps)) keeps the whole chain in
                the natural_log_exp activation-table set (no sqrt-set
                switch, no slow DVE reciprocal).

                `warm` > 0 issues dummy ones-matmuls into a scratch PSUM
                bank while the stats chain runs — the PE p-state drops to
                1.2GHz after a ~3.4us idle gap, so keeping it streaming
                through this serial chain makes the next phase's matmuls
                start at full clock."""
                if warm:
                    w_ps = ps.tile([1, S], F32, tag="warm", name="warm")
                    for w in range(warm):
                        nc.tensor.matmul(w_ps[:], ones128p[:], y[:, w % KT, :],
                                         start=True, stop=True)
                sums = sb.tile([1, S], ADT, tag="sums", name="sums", bufs=2)
                nc.vector.tensor_copy(sums[:], st_ps[:, 0, :])
                # s2h = (sum/sqrt(H))^2 straight from PSUM on Act
                s2h = sb.tile([1, S], F32, tag="s2h", name="s2h", bufs=2)
                nc.scalar.activation(s2h[:], st_ps[:, 0, :], AF.Square,
                                     scale=RSQH)
                varh = sb.tile([1, S], F32, tag="varh", name="varh", bufs=2)
                nc.vector.tensor_sub(varh[:], st_ps[:, 1, :], s2h[:])
                lnv = sb.tile([1, S], F32, tag="lnv", name="lnv", bufs=2)
                nc.scalar.activation(lnv[:], varh[:], AF.Ln,
                                     bias=eps_t[:], scale=1.0 / H)
                rstd = sb.tile([1, S], ADT, tag="rstd", name="rstd", bufs=2)
                nc.scalar.activation(rstd[:], lnv[:], AF.Exp, scale=-0.5)
                mub_ps = ps.tile([128, S], F32, tag="mub", name="mub", bufs=1)
                nc.tensor.matmul(mub_ps[:], invh128c[:], sums[:],
                                 start=True, stop=True)
                rsb_ps = ps.tile([128, S], F32, tag="rsb", name="rsb", bufs=1)
                nc.tensor.matmul(rsb_ps[:], ones128c[:], rstd[:],
                                 start=True, stop=True)
                if warm:
                    for w in range(warm):
                        nc.tensor.matmul(w_ps[:], ones128p[:], y[:, w % KT, :],
                                         start=True, stop=True)
                # bf16 SBUF copies of the broadcasts: the GPSIMD half needs
                # SBUF (no PSUM port), and bf16 operands put the DVE
                # tensor_tensor ops in 2x mode (424ns vs 690ns a tile).
                # Mean/rstd in bf16 cost ~1e-3 relative — well inside
                # tolerance. DVE tile 0 reads PSUM directly so the first
                # output tile doesn't wait for the copies.
                mub_sb = sb.tile([128, S], ADT, tag="mubsb", name="mubsb", bufs=2)
                nc.scalar.copy(mub_sb[:], mub_ps[:])
                rsb_sb = sb.tile([128, S], ADT, tag="rsbsb", name="rsbsb", bufs=2)
                nc.scalar.copy(rsb_sb[:], rsb_ps[:])
                for k in range(KT):
                    dve = k % 3 != 2
                    eng = nc.vector if dve else nc.gpsimd
                    first = dve and k == 0
                    mu_in = mub_ps[:] if first else mub_sb[:]
                    rs_in = rsb_ps[:] if first else rsb_sb[:]
                    t1 = sb.tile([128, S], F32 if first else ADT, tag="lnt1",
                                 name="lnt1", bufs=4)
                    eng.tensor_sub(t1[:], y[:, k, :], mu_in)
                    t2 = sb.tile([128, S], ADT, tag="lnt2", name="lnt2", bufs=4)
                    eng.tensor_mul(t2[:], t1[:], rs_in)
                    nc.scalar.activation(out[:, k, :], t2[:], AF.Identity,
                                         bias=bet[:, l, k:k + 1],
                                         scale=gam[:, l, k:k + 1])
                    if preload_gelu and k == 0:
                        # dummy 1-elem gelu right after the first normalize
                        # identity: the gelu-set ACT_TABLE_LOAD lands in the
                        # Act engine's slack between identity tiles (paced
                        # by the DVE/gpsimd normalize) instead of on the
                        # FFN critical path. Later identities exist in the
                        # gelu set too, so no reload.
                        scr = sb.tile([1, 1], F32, tag="scr", name="scr",
                                      bufs=2)
                        nc.scalar.activation(scr[:], eps_t[:], AF.Gelu)

            def load_w768(dram, l, k, name):
                w = wpool.tile([128, H], WDT, tag="w768", name=name)
                nc.sync.dma_start(out=w[:], in_=dram.ap()[l, bass.ts(k, 128), :])
                return w

            def layer_body(l, last=False):
                # ============ QKV + attention, pair-pipelined ============
                # Q is projected first in two k-outer groups of 3 m-tiles
                # (3 PSUM banks): the k-outer order lets each matmul start
                # as soon as the previous layer's LN2 normalize emits that
                # xT k-tile, filling the LN2 chain's PE-idle window.
                # Head-pair m then needs exactly the m-th K-projection
                # tile, so K is computed per-pair inside the softmax
                # pipeline: while the Act engine streams exp tiles for
                # pair p+1, the PE runs K_{p+1} and PV_p.
                wq = [load_w768(WQ, l, k, f"wq{k}") for k in range(KT)]
                wk = [load_w768(WK, l, k, f"wk{k}") for k in range(KT)]
                wv = [load_w768(WV, l, k, f"wv{k}") for k in range(KT)]

                def proj_qk(ps, m):
                    p_q = ps.tile([128, S], F32, tag="qk", name="pq", bufs=2)
                    for k in range(KT):
                        nc.tensor.matmul(p_q[:], wq[k][:, bass.ts(m, 128)],
                                         xT[:, k, :], start=(k == 0),
                                         stop=(k == KT - 1))
                    nc.scalar.activation(qT[:, m, :], p_q[:], AF.Identity,
                                         bias=bq_t[:, l, m:m + 1])
                    p_k = ps.tile([128, S], F32, tag="qk", name="pk", bufs=2)
                    for k in range(KT):
                        nc.tensor.matmul(p_k[:], wk[k][:, bass.ts(m, 128)],
                                         xT[:, k, :], start=(k == 0),
                                         stop=(k == KT - 1))
                    nc.scalar.activation(kTt[:, m, :], p_k[:], AF.Identity,
                                         bias=bk_t[:, l, m:m + 1])

                def scores_exp(ps, pr):
                    exps = {}
                    for hh in range(2):
                        p0 = hh * DH
                        tp = None if hh == 0 else (64, 0)
                        for m in range(ST):
                            s_ps = ps.tile([128, S], F32, tag="scores",
                                           name="sps", bufs=2)
                            nc.tensor.matmul(
                                s_ps[:],
                                kTt[p0:p0 + DH, pr, bass.ts(m, 128)],
                                qT[p0:p0 + DH, pr, :],
                                start=True, stop=True, tile_position=tp)
                            e_t = sb.tile([128, S], ADT, tag="exp",
                                          name="expt", bufs=16)
                            nc.scalar.activation(e_t[:], s_ps[:], AF.Exp,
                                                 bias=ext[:, m:m + 1],
                                                 scale=SCALE)
                            exps[(hh, m)] = e_t
                    return exps

                def pv_split(ps, pr, exps):
                    # Last-pair variant: per-head denominator copy/recip so
                    # head 0's chain runs while head 1's PV matmuls are
                    # still accumulating — shortens the attention tail the
                    # Wo phase waits on.
                    c2 = ps.tile([128, 2, S], F32, tag="ctx", name="cps",
                                 bufs=2)
                    c = RECIP_APPROX_FAST_CONSTS
                    rcp2 = sb.tile([1, 2, S], ADT, tag="rcp2", name="rcp2",
                                   bufs=3)
                    for hh in range(2):
                        h = 2 * pr + hh
                        for m in range(ST):
                            nc.tensor.matmul(c2[0:DH + 1, hh, :],
                                             v_aug[:, m, h, :],
                                             exps[(hh, m)][:],
                                             start=(m == 0), stop=(m == ST - 1))
                        den = sb.tile([1, S], F32, tag="den1", name="den1",
                                      bufs=2)
                        nc.vector.tensor_copy(den[:], c2[DH:DH + 1, hh, :])
                        nc.vector._custom_dve(
                            RECIPROCAL_APPROX_FAST, out=rcp2[:, hh, :],
                            in0=den[:], s0=c["s0"], s1=c["s1"],
                            imm2=c["imm2"])
                    return (pr, c2, rcp2)

                def pv(ps, pr, exps):
                    # Both heads' PV into one 2-bank PSUM tile so the two
                    # softmax denominator rows (partition 64) sit in one
                    # contiguous free range: one DVE copy to SBUF, then the
                    # custom-DVE fast reciprocal (bf16 out). The custom op
                    # reads garbage from PSUM on real HW (SBUF-only), hence
                    # the copy.
                    c2 = ps.tile([128, 2, S], F32, tag="ctx", name="cps",
                                 bufs=2)
                    for hh in range(2):
                        h = 2 * pr + hh
                        for m in range(ST):
                            nc.tensor.matmul(c2[0:DH + 1, hh, :],
                                             v_aug[:, m, h, :],
                                             exps[(hh, m)][:],
                                             start=(m == 0), stop=(m == ST - 1))
                    den = sb.tile([1, 2, S], F32, tag="den", name="den",
                                  bufs=3)
                    nc.vector.tensor_copy(den[:], c2[DH:DH + 1, :, :])
                    rcp2 = sb.tile([1, 2, S], ADT, tag="rcp2", name="rcp2",
                                   bufs=3)
                    c = RECIP_APPROX_FAST_CONSTS
                    nc.vector._custom_dve(
                        RECIPROCAL_APPROX_FAST,
                        out=rcp2[:].rearrange("p a s -> p (a s)"),
                        in0=den[:].rearrange("p a s -> p (a s)"),
                        s0=c["s0"], s1=c["s1"], imm2=c["imm2"])
                    return (pr, c2, rcp2)

                def att_finalize(ps, pend):
                    # bc_ps shares the scores pool rotation (no extra bank)
                    pr_, c2_, rcp2_ = pend
                    bc_ps = ps.tile([128, S], F32, tag="scores", name="bcps",
                                    bufs=2)
                    for hh in range(2):
                        nc.tensor.matmul(bc_ps[hh * DH:(hh + 1) * DH, :],
                                         ones64[:], rcp2_[:, hh, :],
                                         start=True, stop=True)
                    bc_sb = sb.tile([128, S], ADT, tag="bcsb", name="bcsb",
                                    bufs=2)
                    nc.vector.tensor_copy(bc_sb[:], bc_ps[:])
                    for hh in range(2):
                        p0 = hh * DH
                        nc.vector.tensor_mul(
                            qT[p0:p0 + DH, pr_, :],
                            c2_[0:DH, hh, :],
                            bc_sb[p0:p0 + DH, :])

                with tc.tile_pool(name="ps_qk", bufs=1, space="PSUM") as psqk:
                    # pair 0 projections + scores, then V while its exps run
                    proj_qk(psqk, 0)
                    with tc.tile_pool(name="ps_sc", bufs=1,
                                      space="PSUM") as pssc:
                        exps0 = scores_exp(pssc, 0)
                        # V projection (seq-major, bias via K=1 ones matmul)
                        bv_row = sb.tile([1, H], ADT, tag="bvrow",
                                         name="bvrow", bufs=2)
                        nc.sync.dma_start(out=bv_row[:], in_=BVB.ap()[l:l + 1, :])
                        with tc.tile_pool(name="ps_v", bufs=1,
                                          space="PSUM") as psv:
                            for s in range(ST):
                                p_a = psv.tile([128, S], F32, tag="va",
                                               name="pva", bufs=2)
                                p_b = psv.tile([128, 256], F32, tag="vb",
                                               name="pvb", bufs=2)
                                for k in range(KT):
                                    nc.tensor.matmul(
                                        p_a[:], xT[:, k, bass.ts(s, 128)],
                                        wv[k][:, 0:512],
                                        start=(k == 0), stop=False)
                                    nc.tensor.matmul(
                                        p_b[:], xT[:, k, bass.ts(s, 128)],
                                        wv[k][:, 512:768],
                                        start=(k == 0), stop=False)
                                nc.tensor.matmul(p_a[:], ones128pc[:],
                                                 bv_row[:, 0:512],
                                                 start=False, stop=True)
                                nc.tensor.matmul(p_b[:], ones128pc[:],
                                                 bv_row[:, 512:768],
                                                 start=False, stop=True)
                                nc.vector.tensor_copy(
                                    v_aug[:, s, 0:8, 0:DH],
                                    p_a[:].rearrange("p (h c) -> p h c", c=DH))
                                nc.vector.tensor_copy(
                                    v_aug[:, s, 8:12, 0:DH],
                                    p_b[:].rearrange("p (h c) -> p h c", c=DH))
                        # steady pair pipeline
                        with tc.tile_pool(name="ps_ctx", bufs=1,
                                          space="PSUM") as psctx:
                            pending = None
                            exps = exps0
                            for pr in range(NP):
                                if pr + 1 < NP:
                                    proj_qk(psqk, pr + 1)
                                # finalize pair pr-1 now: its reciprocal was
                                # issued a full pair-iteration ago, so the
                                # broadcast matmuls at the PE queue head
                                # never wait on the DVE chain — and its
                                # scores-pool slot was drained by exps(pr)
                                # long ago.
                                if pending is not None:
                                    att_finalize(pssc, pending)
                                nxt = (scores_exp(pssc, pr + 1)
                                       if pr + 1 < NP else None)
                                pending = pv_split(psctx, pr, exps)
                                exps = nxt
                            # last pair: per-head broadcast/normalize so
                            # head 0's qT tile lands while head 1's chain
                            # still runs (Wo waits on this tail).
                            pr_, c2_, rcp2_ = pending
                            bc_ps = pssc.tile([128, S], F32, tag="scores",
                                              name="bcps", bufs=2)
                            bc_sb = sb.tile([128, S], ADT, tag="bcsb",
                                            name="bcsb", bufs=2)
                            for hh in range(2):
                                p0 = hh * DH
                                nc.tensor.matmul(bc_ps[p0:p0 + DH, :],
                                                 ones64[:], rcp2_[:, hh, :],
                                                 start=True, stop=True)
                                nc.vector.tensor_copy(bc_sb[p0:p0 + DH, :],
                                                      bc_ps[p0:p0 + DH, :])
                                nc.vector.tensor_mul(
                                    qT[p0:p0 + DH, pr_, :],
                                    c2_[0:DH, hh, :],
                                    bc_sb[p0:p0 + DH, :])

                if KDBG and l == 0:
                    dq = pers.tile([128, KT, S], F32, tag="dbgq")
                    nc.vector.tensor_copy(dq[:], qT[:])
                    nc.sync.dma_start(
                        out=DQ.ap().rearrange("(k p) s -> p k s", p=128),
                        in_=dq[:])
                    dk = pers.tile([128, KT, S], F32, tag="dbgk")
                    nc.vector.tensor_copy(dk[:], kTt[:])
                    nc.sync.dma_start(
                        out=DK.ap().rearrange("(k p) s -> p k s", p=128),
                        in_=dk[:])

                # ================= Wo + residual + LN1 =================
                with tc.tile_pool(name="ps_wo", bufs=1, space="PSUM") as ps:
                    wo = [load_w768(WO, l, k, f"wo{k}") for k in range(KT)]
                    st_ps = ps.tile([1, 2, S], F32, tag="sum", name="sum")
                    for m in range(MT):
                        p_o = ps.tile([128, S], F32, tag="proj", name="po",
                                      bufs=3)
                        for k in range(KT):
                            nc.tensor.matmul(p_o[:], wo[k][:, bass.ts(m, 128)],
                                             qT[:, k, :], start=(k == 0),
                                             stop=(k == KT - 1))
                        # y = (psum + bo) + x   (into kTt, reused as y)
                        nc.vector.scalar_tensor_tensor(
                            kTt[:, m, :], p_o[:], bo_t[:, l, m:m + 1],
                            xT[:, m, :], op0=ALU.add, op1=ALU.add)
                        ln_sums(ps, kTt, m, m == 0, m == MT - 1, st_ps)
                    ln_finalize(ps, kTt, g1_t, b1_t, l, attnT, st_ps, warm=6)

                # ================= FFN =================
                with tc.tile_pool(name="ps_ffn", bufs=1, space="PSUM") as ps:
                    ffo = [ps.tile([128, S], F32, tag="ffo", name=f"ffo{m}",
                                   bufs=6)
                           for m in range(MT)]
                    # software-pipelined: p_f chain of ko+1 issues before the
                    # ffo accumulation of ko so the PE isn't starved while the
                    # gelu of ko drains.
                    ffts = {}
                    pf01 = []

                    def ffn1_weights(ko):
                        wi_t = wf1pool.tile([128, KT, 128], WDT, tag="wff1",
                                            name=f"wi{ko}")
                        nc.sync.dma_start(out=wi_t[:], in_=WI.ap()[l, ko])
                        wf_t = wpool.tile([128, H], WDT, tag="w768",
                                          name=f"wf{ko}")
                        nc.sync.dma_start(out=wf_t[:],
                                          in_=WF.ap()[l, bass.ts(ko, 128), :])
                        return wi_t, wf_t

                    def ffn1_finish(ko, wi_t, wf_t, p_f):
                        ff_t = sb.tile([128, S], ADT, tag="fft", name="fft",
                                       bufs=4)
                        nc.scalar.activation(ff_t[:], p_f[:], AF.Gelu,
                                             bias=bi_t[:, l, ko:ko + 1])
                        ffts[ko] = (ff_t, wf_t)

                    for ko in range(IT + 1):
                        if ko < IT:
                            wi_t, wf_t = ffn1_weights(ko)
                            p_f = ps.tile([128, S], F32, tag="ff1", name="pf",
                                          bufs=2)
                            for k in range(KT):
                                nc.tensor.matmul(p_f[:], wi_t[:, k, :],
                                                 attnT[:, k, :],
                                                 start=(k == 0),
                                                 stop=(k == KT - 1))
                            ffn1_finish(ko, wi_t, wf_t, p_f)
                        if ko >= 1:
                            ff_p, wf_p = ffts.pop(ko - 1)
                            for m in range(MT):
                                nc.tensor.matmul(ffo[m][:],
                                                 wf_p[:, bass.ts(m, 128)],
                                                 ff_p[:], start=(ko - 1 == 0),
                                                 stop=(ko - 1 == IT - 1))
                    # dummy 1-elem ln: hoists the natural_log_exp-set
                    # ACT_TABLE_LOAD off LN2's serial chain (Act is idle
                    # after the last gelu while FFN2 drains).
                    scr2 = sb.tile([1, 1], F32, tag="scr", name="scr2", bufs=2)
                    nc.scalar.activation(scr2[:], eps_t[:], AF.Ln)
                    for m in range(MT):
                        # y2 = (ffo + bf) + attnT   (into kTt)
                        nc.vector.scalar_tensor_tensor(
                            kTt[:, m, :], ffo[m][:], bf_t[:, l, m:m + 1],
                            attnT[:, m, :], op0=ALU.add, op1=ALU.add)
                with tc.tile_pool(name="ps_ln2", bufs=1, space="PSUM") as ps:
                    st_ps = ps.tile([1, 2, S], F32, tag="sum", name="sum")
                    ln_sums_split(ps, kTt, st_ps)
                    # On the last pass the LN2 normalize writes the fp32
                    # output tile directly (skips a 3us full-width copy).
                    ln_finalize(ps, kTt, g2_t, b2_t, l,
                                xout if last else xT, st_ps, warm=6)

            xout = pers.tile([128, KT, S], F32, tag="xout")
            for r in range(repeat):
                for l in range(n_layers):
                    layer_body(l, last=(r == repeat - 1 and l == n_layers - 1))

            # per-k-tile output DMA: each slice ships as soon as its
            # final LN2 normalize identity lands, instead of waiting for
            # the whole tile.
            outr = OUT.ap().rearrange("(k p) s -> p k s", p=128)
            for k in range(KT):
                nc.sync.dma_start(out=outr[:, k, :], in_=xout[:, k, :])

    nc.compile()
    return nc


_CACHE = {}


def get_program(repeat=1, n_layers=L):
    key = (repeat, n_layers)
    if key not in _CACHE:
        _CACHE[key] = build_program(repeat, n_layers)
    return _CACHE[key]


def make_input_maps(inputs):
    """Per-core input maps from the full-batch input dict."""
    import ml_dtypes
    wnp = ml_dtypes.bfloat16 if WDT == BF16 else np.float32
    anp = ml_dtypes.bfloat16 if ADT == BF16 else np.float32
    hs = np.ascontiguousarray(np.asarray(inputs["hidden_states"], np.float32))
    mask = np.asarray(inputs["attention_mask"], np.float32)
    wi = np.ascontiguousarray(
        np.asarray(inputs["Wi"], np.float32).reshape(L, KT, 128, IT, 128)
        .transpose(0, 3, 2, 1, 4)).astype(wnp)
    shared = {
        "WQ": np.ascontiguousarray(np.asarray(inputs["Wq"], np.float32)).astype(wnp),
        "WK": np.ascontiguousarray(np.asarray(inputs["Wk"], np.float32)).astype(wnp),
        "WV": np.ascontiguousarray(np.asarray(inputs["Wv"], np.float32)).astype(wnp),
        "WO": np.ascontiguousarray(np.asarray(inputs["Wo"], np.float32)).astype(wnp),
        "WI": wi,
        "WF": np.ascontiguousarray(np.asarray(inputs["Wf"], np.float32)).astype(wnp),
        "BVB": np.asarray(inputs["bv"], np.float32).astype(anp),
        "PP8": np.ascontiguousarray(np.stack(
            [np.asarray(inputs[k], np.float32).reshape(L, KT, 128)
             .transpose(2, 0, 1)
             for k in ("bq", "bk", "bo", "bf", "ln1_g", "ln1_b",
                       "ln2_g", "ln2_b")], axis=1)),
        "PBI": np.ascontiguousarray(
            np.asarray(inputs["bi"], np.float32).reshape(L, IT, 128)
            .transpose(2, 0, 1)),
    }
    in_maps = []
    for c in range(B):
        ext = ((1.0 - mask[c]) * -10000.0).astype(np.float32).reshape(ST, 128)
        in_maps.append({
            "XT": np.ascontiguousarray(hs[c].T).astype(anp),
            "EXTM": ext,
            **shared,
        })
    return in_maps


def kernel(**inputs):
    nc = get_program(repeat=1)
    in_maps = make_input_maps(inputs)
    res = run_bass_kernel_spmd(nc, in_maps, list(range(B)))
    out = np.stack([res.results[c]["OUT"].T for c in range(B)], axis=0)
    return out.astype(np.float32)
